# revision 1
# baseline (speedup 1.0000x reference)
"""MoE logistic regression kernel for 8 Trainium2 NeuronCores.

Math (after dead-code elimination of the reference's unused router path):
    noise_logits = x @ noise_w.T + noise_b            # [B, E]
    top8 = top_k(noise_logits, 8)
    gates = softmax over the top-8 entries (others 0)
    expert = sigmoid(x @ expert_w.T + expert_b)       # [B, E]
    out[b] = sum_e gates[b,e] * expert[b,e]           # [B, 1]

Sharding: batch split 8 ways (2048 rows/core); weights replicated.

Key implementation choices:
- x is transposed on the host so each core streams contiguous [D, BC]
  chunks with D on partitions; no on-chip transpose of x.
- x and w are split into fp16 (hi, lo) pairs on the host (exact to ~22
  mantissa bits). The matmul runs 3 fp16 passes (hi@wh + lo@wh + hi@wl)
  accumulating in fp32 PSUM: ~fp32 accuracy at 3/4 the fp32 PE cost.
  (The top-8 selection margins require ~1e-6 logit accuracy: the
  smallest 8th/9th gap over the whole fixed batch is 8.8e-6.)
- noise_w/expert_w are concatenated into one 128-wide stationary operand
  so x streams through the PE once per (chunk, pass) for both matmuls;
  biases are added per-partition by the ACT epilogue ops.
- top-8 per row via the DVE Max8 + MatchReplace8 instructions; gates via
  exp(v - m1) with the (e_all - e_zap) trick which is exactly zero off
  the top-8; final dot + 1/Z normalization per 128-row tile.
"""

import sys

import numpy as np

if "/opt/trn_rl_repo" not in sys.path:
    sys.path.insert(0, "/opt/trn_rl_repo")

B, D, E, TOPK, NCORES = 16384, 4096, 64, 8, 8
BC = B // NCORES      # batch rows per core
BT = 512              # batch tile (one PSUM bank of fp32)
NT = BC // BT         # batch tiles per core
NK = D // 128         # contraction chunks
NEG_BIG = -1e30

_cached = {}


def _build_program(mm_dtype="fp16x2"):
    import concourse.bass as bass
    import concourse.tile as tile
    from concourse import bacc, mybir
    from concourse.masks import make_identity

    f32 = mybir.dt.float32
    f16 = mybir.dt.float16
    split = mm_dtype == "fp16x2"
    wdt = f16 if split else getattr(mybir.dt, mm_dtype)
    act = mybir.ActivationFunctionType

    nc = bacc.Bacc("TRN2", target_bir_lowering=False, debug=False)
    if split:
        # x as fp16 (hi, lo): [D, NT, 2, BT]; w pair pre-swizzled so the
        # SBUF image [128, NK*2*128] is one contiguous DMA.
        xt = nc.dram_tensor("xt", [D, NT, 2, BT], f16, kind="ExternalInput").ap()
        wt = nc.dram_tensor("wt", [128, NK * 2 * 128], f16,
                            kind="ExternalInput").ap()
    else:
        xt = nc.dram_tensor("xt", [D, NT, BT], f32, kind="ExternalInput").ap()
        wt = nc.dram_tensor("wt", [128, NK * 128], f32, kind="ExternalInput").ap()
    bb = nc.dram_tensor("bb", [128, 1], f32, kind="ExternalInput").ap()
    out = nc.dram_tensor("out", [BC, 1], f32, kind="ExternalOutput").ap()

    with tile.TileContext(nc) as tc:
        with (
            tc.tile_pool(name="consts", bufs=1) as consts,
            tc.tile_pool(name="xpool", bufs=6) as xpool,
            tc.tile_pool(name="eppool", bufs=4) as eppool,
            tc.tile_pool(name="small", bufs=3) as small,
            tc.tile_pool(name="psacc", bufs=1, space=bass.MemorySpace.PSUM) as psacc,
            tc.tile_pool(name="pstr", bufs=2, space=bass.MemorySpace.PSUM) as pstr,
            tc.tile_pool(name="psfin", bufs=1, space=bass.MemorySpace.PSUM) as psfin,
        ):
            # ---- constants ----
            if split:
                wt_first = consts.tile([128, 2, 2, 128], wdt)
                nc.scalar.dma_start(out=wt_first, in_=wt[:, 0:2 * 2 * 128]
                                    .rearrange("p (nk two m) -> p nk two m",
                                               nk=2, two=2))
                wt_sb = consts.tile([128, NK - 2, 2, 128], wdt)
                nc.scalar.dma_start(out=wt_sb, in_=wt[:, 2 * 2 * 128:]
                                    .rearrange("p (nk two m) -> p nk two m",
                                               nk=NK - 2, two=2))
            else:
                wt_sb = consts.tile([128, NK, 128], wdt)
                nc.scalar.dma_start(out=wt_sb, in_=wt)
            bb_sb = consts.tile([128, 1], f32)
            nc.scalar.dma_start(out=bb_sb, in_=bb)
            ident = consts.tile([128, 128], f32)
            make_identity(nc, ident)
            # warm the ACT function tables during the DMA/matmul phase so the
            # first epilogue ops don't pay serial LoadActFuncSet latency
            warm = consts.tile([1, 1], f32)
            nc.vector.memset(warm, 0.0)
            nc.scalar.add(warm, warm, bb_sb[0:1, :])
            nc.scalar.activation(warm, warm, func=act.Sigmoid,
                                 bias=bb_sb[0:1, :])
            nc.scalar.activation(warm, warm, func=act.Exp)
            nc.scalar.mul(warm, warm, 1.0)
            final_sb = consts.tile([128, NT * 4], f32)

            # ---- matmuls: acc[t][0:64,:] = noise logits.T (pre-bias),
            #               acc[t][64:128,:] = expert logits.T (pre-bias)
            accs = [psacc.tile([128, BT], f32, tag=f"acc{t}", name=f"acc{t}")
                    for t in range(NT)]
            if split:
                # pair k-chunks: one 2MB DMA covers chunks 2kk and 2kk+1
                xview = xt.rearrange("(nkk two p) nt t b -> nkk p two nt t b",
                                     p=128, two=2)
                for kk in range(NK // 2):
                    xk = xpool.tile([128, 2, NT, 2, BT], wdt, tag="xk")
                    nc.sync.dma_start(out=xk, in_=xview[kk])
                    for c in range(2):
                        k = 2 * kk + c
                        wsrc = wt_first if k < 2 else wt_sb
                        ki = k if k < 2 else k - 2
                        wh = wsrc[:, ki, 0, :]
                        wl = wsrc[:, ki, 1, :]
                        for t in range(NT):
                            nc.tensor.matmul(accs[t], lhsT=wh,
                                             rhs=xk[:, c, t, 0, :],
                                             start=(k == 0), stop=False)
                            nc.tensor.matmul(accs[t], lhsT=wh,
                                             rhs=xk[:, c, t, 1, :],
                                             start=False, stop=False)
                            nc.tensor.matmul(accs[t], lhsT=wl,
                                             rhs=xk[:, c, t, 0, :],
                                             start=False,
                                             stop=(k == NK - 1))
            else:
                xview = xt.rearrange("(nk p) nt b -> nk p nt b", p=128)
                for k in range(NK):
                    xk = xpool.tile([128, NT, BT], wdt, tag="xk")
                    nc.sync.dma_start(out=xk, in_=xview[k])
                    for t in range(NT):
                        nc.tensor.matmul(accs[t], lhsT=wt_sb[:, k, :],
                                         rhs=xk[:, t, :],
                                         start=(k == 0), stop=(k == NK - 1))

            # ---- epilogue: pass 1 emits all bias/sigmoid + transposes so
            # the ACT FIFO isn't blocked by tile t's exp stream when tile
            # t+1's head ops become ready; pass 2 does the per-tile math.
            ps_nes = []
            for t in range(NT):
                noiseT = eppool.tile([64, BT], f32, tag="noiseT")
                nc.scalar.add(noiseT, accs[t][0:64, :], bb_sb[0:64, :])
                eoT = eppool.tile([64, BT], f32, tag="eoT")
                nc.scalar.activation(eoT, accs[t][64:128, :],
                                     func=act.Sigmoid, bias=bb_sb[64:128, :])
                # transpose to batch-major: [128 batch, j | 4+j, 64]
                ps_ne = pstr.tile([128, 8, 64], f32, tag="ps_ne",
                                  name=f"ps_ne{t}")
                for j in range(4):
                    nc.tensor.transpose(ps_ne[:, j, :],
                                        noiseT[:, j * 128:(j + 1) * 128],
                                        ident[0:64, 0:64])
                    nc.tensor.transpose(ps_ne[:, 4 + j, :],
                                        eoT[:, j * 128:(j + 1) * 128],
                                        ident[0:64, 0:64])
                ps_nes.append(ps_ne)
            for t in range(NT):
                ps_ne = ps_nes[t]
                e_all = small.tile([128, 4, 64], f32, tag="e_all")
                e_zap = small.tile([128, 4, 64], f32, tag="e_zap")
                zsum = small.tile([128, 4], f32, tag="zsum")
                for j in range(4):
                    v = ps_ne[:, j, :]
                    tv = small.tile([128, 8], f32, tag="tv")
                    nc.vector.max(tv, v)                      # top-8, descending
                    zap = small.tile([128, 64], f32, tag="zap")
                    nc.vector.match_replace(out=zap, in_to_replace=tv,
                                            in_values=v, imm_value=NEG_BIG)
                    negm1 = small.tile([128, 1], f32, tag="negm1")
                    nc.scalar.mul(negm1, tv[:, 0:1], -1.0)
                    nc.scalar.activation(e_all[:, j, :], v, func=act.Exp,
                                         bias=negm1)
                    nc.scalar.activation(e_zap[:, j, :], zap, func=act.Exp,
                                         bias=negm1)
                # g = exp(v-m1) on top-8 positions, exactly 0 elsewhere;
                # grouped DVE math over all four 128-row subtiles at once
                g = small.tile([128, 4, 64], f32, tag="g")
                nc.vector.tensor_sub(g, e_all, e_zap)
                nc.vector.reduce_sum(zsum, g, axis=mybir.AxisListType.X)
                scr = small.tile([128, 4, 64], f32, tag="scr")
                nc.vector.tensor_mul(scr, g, ps_ne[:, 4:8, :])
                s4 = small.tile([128, 4], f32, tag="s4")
                nc.vector.reduce_sum(s4, scr, axis=mybir.AxisListType.X)
                rz = small.tile([128, 4], f32, tag="rz")
                nc.vector.reciprocal(rz, zsum)
                nc.vector.tensor_mul(final_sb[:, t * 4:(t + 1) * 4], s4, rz)

            # ---- output: [128, 16] -> [16, 128] -> DRAM [2048, 1] ----
            fin_ps = psfin.tile([16, 128], f32, tag="fin")
            nc.tensor.transpose(fin_ps, final_sb, ident)
            fin_t = eppool.tile([16, 128], f32, tag="fint")
            nc.scalar.copy(fin_t, fin_ps)
            nc.sync.dma_start(out=out.rearrange("(c p) o -> c (p o)", p=128),
                              in_=fin_t)

    nc.compile()
    return nc


def get_program(mm_dtype="fp16x2"):
    if mm_dtype not in _cached:
        _cached[mm_dtype] = _build_program(mm_dtype)
    return _cached[mm_dtype]


def make_in_maps(x, noise_w, noise_b, expert_w, expert_b, mm_dtype="fp16x2"):
    """Host-side sharding: per-core transposed x slice + replicated weights."""
    w_comb = np.concatenate([noise_w, expert_w], axis=0).astype(np.float32)  # [128, D]
    wt32 = np.ascontiguousarray(w_comb.T)                                    # [D, 128]
    bb = np.concatenate([noise_b, expert_b]).astype(np.float32).reshape(128, 1)
    if mm_dtype == "fp16x2":
        wh = wt32.astype(np.float16)
        wl = (wt32 - wh.astype(np.float32)).astype(np.float16)
        wp = np.stack([wh, wl], axis=1)                   # [D, 2, 128]
        # SBUF image: partition p holds [nk, 2, 128] for rows nk*128+p
        wt = np.ascontiguousarray(
            wp.reshape(NK, 128, 2, 128).transpose(1, 0, 2, 3).reshape(128, -1))
    else:
        wt = np.ascontiguousarray(
            wt32.reshape(NK, 128, 128).transpose(1, 0, 2).reshape(128, -1))
    in_maps = []
    for c in range(NCORES):
        xs = np.ascontiguousarray(x[c * BC:(c + 1) * BC, :].T)               # [D, BC]
        if mm_dtype == "fp16x2":
            xh = xs.astype(np.float16)
            xl = (xs - xh.astype(np.float32)).astype(np.float16)
            xs = np.ascontiguousarray(
                np.stack([xh.reshape(D, NT, BT), xl.reshape(D, NT, BT)],
                         axis=2))                                            # [D,NT,2,BT]
        else:
            xs = np.ascontiguousarray(xs.reshape(D, NT, BT))
        in_maps.append({"xt": xs, "wt": wt, "bb": bb})
    return in_maps


def kernel(x, noise, router_w, router_b, noise_w, noise_b, expert_w, expert_b,
           _trace=False):
    from concourse.bass_utils import run_bass_kernel_spmd

    x = np.asarray(x, dtype=np.float32)
    nc = get_program()
    in_maps = make_in_maps(x, np.asarray(noise_w), np.asarray(noise_b),
                           np.asarray(expert_w), np.asarray(expert_b))
    res = run_bass_kernel_spmd(nc, in_maps, core_ids=list(range(NCORES)),
                               trace=_trace)
    out = np.concatenate([r["out"] for r in res.results], axis=0)
    if _trace:
        kernel.last_results = res
    return out



# revision 17
# speedup vs baseline: 2.3182x; 2.3182x over previous
"""MoE logistic regression kernel for 8 Trainium2 NeuronCores.

Math (after dead-code elimination of the reference's unused router path):
    noise_logits = x @ noise_w.T + noise_b            # [B, E]
    top8 = top_k(noise_logits, 8)
    gates = softmax over the top-8 entries (others 0)
    expert = sigmoid(x @ expert_w.T + expert_b)       # [B, E]
    out[b] = sum_e gates[b,e] * expert[b,e]           # [B, 1]

Sharding: batch split 8 ways (2048 rows/core); weights replicated.

Implementation notes:
- Single-pass fp16 matmul (x and w rounded to fp16 on the host; fp32
  PSUM accumulate). Logit error ~4e-4 flips the top-8 set on only ~25
  of 16384 rows whose 8th/9th margin is that small; output l2 rel err
  ~1.2e-3, far under the 2e-2 gate. Halves DMA traffic and cuts PE
  work 3x vs an exact hi/lo split.
- noise_w/expert_w concatenated into one 128-wide stationary operand so
  x streams through the PE once for both matmuls.
- sigmoid(x) computed as 0.5*tanh(x/2)+0.5 with expert weights/bias
  pre-halved on the host: tanh and exp share one ACT function-table set
  so the kernel needs a single LoadActFuncSet, not 2x1283ns per tile.
- Batch tiles [512,512,512,256,128,128]: the taper keeps every
  epilogue except the last inside the DMA stream's shadow, and the
  final 128-row tile makes the last serial chain short. Tile t's
  epilogue instructions are emitted after tile t+1's first matmul
  chunk so the in-order PE queue never stalls on epilogue deps.
- Each transposed 128-row group gets its OWN PSUM bank: ScalarE and
  VectorE may only touch the same PSUM bank serially, so per-bank
  tiles let exp/tanh (ACT) run concurrently with Max8 (DVE) across
  groups. Top-8 selection is Max8 + an is_ge mask against the 8th
  value; gates and the gate*expert dot use two fused
  tensor_tensor_reduce ops (zsum and 0.5*s4 fall out of their accums).
- Output needs no transpose: out rows c*128+p equal fin16[p, c], and a
  DRAM access pattern rearranged to [p][c] iterates in the same order
  as the SBUF source, so one strided DMA per tile lands rows directly.
- x is staged host-side as [tile, partition, kchunk, col] fp16 so every
  DMA reads contiguous per-partition blocks (full 360GB/s); the last
  tile's trailing chunks shrink so the final matmuls start sooner.
"""

import sys

import numpy as np

if "/opt/trn_rl_repo" not in sys.path:
    sys.path.insert(0, "/opt/trn_rl_repo")

B, D, E, TOPK, NCORES = 16384, 4096, 64, 8, 8
BC = B // NCORES      # batch rows per core
NK = D // 128         # contraction chunks
NEG_BIG = -1e30

TILES = [512, 512, 512, 256, 128, 128]   # batch rows per tile (sum = BC)
NT = len(TILES)
NOUT = BC // 128                         # output columns of fin16

# k-chunks per x DMA, per tile (sum = NK per tile)
CHUNKS = [
    [8, 8, 8, 8],
    [8, 8, 8, 8],
    [8, 8, 8, 8],
    [16, 16],
    [16, 16],
    [16, 8, 4, 2, 1, 1],
]
W_CHUNKS = [8, 8, 8, 8]

_cached = {}


def _build_program():
    import concourse.bass as bass
    import concourse.tile as tile
    from concourse import bacc, mybir
    from concourse.masks import make_identity

    f32 = mybir.dt.float32
    f16 = mybir.dt.float16
    act = mybir.ActivationFunctionType
    alu = mybir.AluOpType

    nc = bacc.Bacc("TRN2", target_bir_lowering=False, debug=False)
    xts = [nc.dram_tensor(f"xt{t}", [128, NK, bt], f16,
                          kind="ExternalInput").ap()
           for t, bt in enumerate(TILES)]
    wt = nc.dram_tensor("wt", [128, NK, 128], f16, kind="ExternalInput").ap()
    bb = nc.dram_tensor("bb", [128, 1], f32, kind="ExternalInput").ap()
    out = nc.dram_tensor("out", [BC, 1], f32, kind="ExternalOutput").ap()

    with tile.TileContext(nc) as tc:
        with (
            tc.tile_pool(name="consts", bufs=1) as consts,
            tc.tile_pool(name="xpool", bufs=8) as xpool,
            tc.tile_pool(name="ep", bufs=2) as ep,
            tc.tile_pool(name="small", bufs=3) as small,
            tc.tile_pool(name="psacc", bufs=2, space=bass.MemorySpace.PSUM) as psacc,
            tc.tile_pool(name="pstr", bufs=5, space=bass.MemorySpace.PSUM) as pstr,
        ):
            # ---- constants ----
            bb_sb = consts.tile([128, 1], f32)
            nc.scalar.dma_start(out=bb_sb, in_=bb)
            wt_sb = consts.tile([128, NK, 128], f16)
            k0 = 0
            for wc in W_CHUNKS:
                nc.scalar.dma_start(out=wt_sb[:, k0:k0 + wc, :],
                                    in_=wt[:, k0:k0 + wc, :])
                k0 += wc
            ident = consts.tile([128, 128], f32)
            make_identity(nc, ident)
            # load the (single) ACT function set during the DMA phase; Tanh
            # and Exp both live in "exp_and_others"
            warm = consts.tile([1, 1], f32)
            nc.vector.memset(warm, 0.0)
            nc.scalar.add(warm, warm, bb_sb[0:1, :])
            nc.scalar.activation(warm, warm, func=act.Tanh)
            nc.scalar.activation(warm, warm, func=act.Exp)

            fin16 = consts.tile([128, NOUT], f32)
            # out rows c*128+p == fin16[p, c]: iterate DRAM as [p][c] and a
            # plain DMA from [128, c] SBUF lands rows with no transpose
            outp = out.rearrange("(c p) o -> p (c o)", p=128)   # [128, NOUT]

            def emit_epilogue(t, acc):
                bt = TILES[t]
                nj = bt // 128
                col0 = sum(TILES[:t]) // 128
                accS = ep.tile([128, bt], f32, tag="accS")
                nc.scalar.add(accS, acc, bb_sb)
                zsum = small.tile([128, nj], f32, tag="zsum")
                s4h = small.tile([128, nj], f32, tag="s4h")
                for j in range(nj):
                    # own PSUM bank per 128-row group: ACT and DVE readers
                    # of different groups may then run concurrently
                    ps = pstr.tile([128, 128], f32, tag="psne",
                                   name=f"psne{t}_{j}")
                    nc.tensor.transpose(ps, accS[:, j * 128:(j + 1) * 128],
                                        ident)
                    v = ps[:, 0:64]
                    tv = small.tile([128, 8], f32, tag="tv")
                    nc.vector.max(tv, v)                  # top-8, descending
                    e_all = small.tile([128, 64], f32, tag="e_all")
                    nc.scalar.activation(e_all, v, func=act.Exp)
                    # exp of the 8th-largest: the top-8 mask threshold moves
                    # to exp-space (monotone), keeping the masking off PSUM
                    t8e = small.tile([128, 1], f32, tag="t8e")
                    nc.scalar.activation(t8e, tv[:, 7:8], func=act.Exp)
                    # expert half holds el/2, so tanh = 2*sigmoid(el)-1
                    th = small.tile([128, 64], f32, tag="th")
                    nc.scalar.activation(th, ps[:, 64:128], func=act.Tanh)
                    # g = exp(v) on the top-8 positions, exactly 0 elsewhere;
                    # zsum and the half-dot fall out of the fused accums
                    g = small.tile([128, 64], f32, tag="g")
                    nc.vector.scalar_tensor_tensor(
                        out=g, in0=e_all, scalar=t8e, in1=e_all,
                        op0=alu.is_ge, op1=alu.mult,
                        accum_out=zsum[:, j:j + 1])
                    scr = small.tile([128, 64], f32, tag="scr")
                    nc.vector.scalar_tensor_tensor(
                        out=scr, in0=g, scalar=0.5, in1=th,
                        op0=alu.mult, op1=alu.mult,
                        accum_out=s4h[:, j:j + 1])
                rz = small.tile([128, nj], f32, tag="rz")
                nc.vector.reciprocal(rz, zsum)
                # sigma = 0.5*tanh+0.5  =>  out = (0.5*s4)/zsum + 0.5
                if nj == 1:
                    nc.vector.tensor_scalar(
                        out=fin16[:, col0:col0 + 1], in0=s4h, scalar1=rz,
                        scalar2=0.5, op0=alu.mult, op1=alu.add)
                else:
                    fr = small.tile([128, nj], f32, tag="fr")
                    nc.vector.tensor_mul(fr, s4h, rz)
                    nc.vector.tensor_scalar(
                        out=fin16[:, col0:col0 + nj], in0=fr,
                        scalar1=0.5, scalar2=None, op0=alu.add)
                # SWDGE: the Pool queue is idle, issues in 25ns, and its
                # fixed cost beats HWDGE by ~1us on the final chain
                nc.gpsimd.dma_start(out=outp[:, col0:col0 + nj],
                                    in_=fin16[:, col0:col0 + nj])

            pending = None
            for t in range(NT):
                bt = TILES[t]
                acc = psacc.tile([128, bt], f32, tag="acc", name=f"acc{t}")
                k = 0
                for c, ck in enumerate(CHUNKS[t]):
                    xk = xpool.tile([128, 16 * 512], f16, tag="xk")
                    xkv = xk.rearrange("p (a b) -> p a b", a=16 * 512 // bt,
                                       b=bt)
                    nc.sync.dma_start(out=xkv[:, 0:ck, :],
                                      in_=xts[t][:, k:k + ck, :])
                    for kc in range(ck):
                        nc.tensor.matmul(acc, lhsT=wt_sb[:, k + kc, :],
                                         rhs=xkv[:, kc, :],
                                         start=(k + kc == 0),
                                         stop=(k + kc == NK - 1))
                    k += ck
                    if c == 0 and pending is not None:
                        emit_epilogue(*pending)
                        pending = None
                pending = (t, acc)
            emit_epilogue(*pending)

    nc.compile()
    return nc


def get_program():
    if "prog" not in _cached:
        _cached["prog"] = _build_program()
    return _cached["prog"]


def make_in_maps(x, noise_w, noise_b, expert_w, expert_b):
    """Host-side sharding: per-core fp16 x slices + replicated fp16 weights.

    The expert weights/bias are halved so the on-chip tanh of the raw
    accumulator equals 2*sigmoid(expert_logit)-1.
    """
    w_comb = np.concatenate([noise_w, 0.5 * np.asarray(expert_w)],
                            axis=0).astype(np.float32)
    wt16 = w_comb.T.astype(np.float16)                       # [D, 128]
    wt = np.ascontiguousarray(
        wt16.reshape(NK, 128, 128).transpose(1, 0, 2))       # [128, NK, 128]
    bb = np.concatenate([noise_b, 0.5 * np.asarray(expert_b)]).astype(
        np.float32).reshape(128, 1)
    in_maps = []
    for c in range(NCORES):
        xs = x[c * BC:(c + 1) * BC, :].astype(np.float16)    # [BC, D]
        xsT = np.ascontiguousarray(xs.T)                     # [D, BC]
        im = {"wt": wt, "bb": bb}
        b0 = 0
        for t, bt in enumerate(TILES):
            # [p, nk, b]: contiguous per-partition blocks per tile
            im[f"xt{t}"] = np.ascontiguousarray(
                xsT[:, b0:b0 + bt].reshape(NK, 128, bt).transpose(1, 0, 2))
            b0 += bt
        in_maps.append(im)
    return in_maps


def kernel(x, noise, router_w, router_b, noise_w, noise_b, expert_w, expert_b,
           _trace=False):
    from concourse.bass_utils import run_bass_kernel_spmd

    x = np.asarray(x, dtype=np.float32)
    nc = get_program()
    in_maps = make_in_maps(x, np.asarray(noise_w), np.asarray(noise_b),
                           np.asarray(expert_w), np.asarray(expert_b))
    res = run_bass_kernel_spmd(nc, in_maps, core_ids=list(range(NCORES)),
                               trace=_trace)
    out = np.concatenate([r["out"] for r in res.results], axis=0)
    if _trace:
        kernel.last_results = res
    return out


# revision 20
# speedup vs baseline: 2.3376x; 1.0084x over previous
"""MoE logistic regression kernel for 8 Trainium2 NeuronCores.

Math (after dead-code elimination of the reference's unused router path):
    noise_logits = x @ noise_w.T + noise_b            # [B, E]
    top8 = top_k(noise_logits, 8)
    gates = softmax over the top-8 entries (others 0)
    expert = sigmoid(x @ expert_w.T + expert_b)       # [B, E]
    out[b] = sum_e gates[b,e] * expert[b,e]           # [B, 1]

Sharding: batch split 8 ways (2048 rows/core); weights replicated.

Implementation notes:
- Single-pass fp16 matmul (x and w rounded to fp16 on the host; fp32
  PSUM accumulate). Logit error ~4e-4 flips the top-8 set on only ~25
  of 16384 rows whose 8th/9th margin is that small; output l2 rel err
  ~1.2e-3, far under the 2e-2 gate. Halves DMA traffic and cuts PE
  work 3x vs an exact hi/lo split.
- noise_w/expert_w concatenated into one 128-wide stationary operand so
  x streams through the PE once for both matmuls.
- sigmoid(x) computed as 0.5*tanh(x/2)+0.5 with expert weights/bias
  pre-halved on the host: tanh and exp share one ACT function-table set
  so the kernel needs a single LoadActFuncSet, not 2x1283ns per tile.
- Batch tiles [512,512,512,256,128,128]: the taper keeps every
  epilogue except the last inside the DMA stream's shadow, and the
  final 128-row tile makes the last serial chain short. Tile t's
  epilogue instructions are emitted after tile t+1's first matmul
  chunk so the in-order PE queue never stalls on epilogue deps.
- Each transposed 128-row group gets its OWN PSUM bank: ScalarE and
  VectorE may only touch the same PSUM bank serially, so per-bank
  tiles let exp/tanh (ACT) run concurrently with Max8 (DVE) across
  groups. Top-8 selection is Max8 plus an is_ge mask applied in
  exp-space (monotone, so thresholding exp(v) against exp(t8) is the
  same selection but keeps the masking off PSUM); gates and the
  gate*expert dot are two fused scalar_tensor_tensor ops whose
  accum_out gives zsum and 0.5*s4 for free.
- Output needs no transpose: out rows c*128+p equal fin16[p, c], and a
  DRAM access pattern rearranged to [p][c] iterates in the same order
  as the SBUF source, so a strided DMA lands rows directly.
- x is staged host-side as [tile, partition, kchunk, col] fp16 so every
  DMA reads contiguous per-partition blocks (full 360GB/s); the last
  tile's trailing chunks shrink so the final matmuls start sooner.
"""

import sys

import numpy as np

if "/opt/trn_rl_repo" not in sys.path:
    sys.path.insert(0, "/opt/trn_rl_repo")

B, D, E, TOPK, NCORES = 16384, 4096, 64, 8, 8
BC = B // NCORES      # batch rows per core
NK = D // 128         # contraction chunks

TILES = [512, 512, 512, 256, 128, 128]   # batch rows per tile (sum = BC)
NT = len(TILES)
NOUT = BC // 128                         # output columns of fin16

# k-chunks per x DMA, per tile (sum = NK per tile)
CHUNKS = [
    [8, 8, 8, 8],
    [8, 8, 8, 8],
    [8, 8, 8, 8],
    [16, 16],
    [16, 16],
    [16, 8, 4, 2, 1, 1],
]
W_CHUNKS = [8, 8, 8, 8]

_cached = {}


def _build_program():
    import concourse.bass as bass
    import concourse.tile as tile
    from concourse import bacc, mybir
    from concourse.masks import make_identity

    f32 = mybir.dt.float32
    f16 = mybir.dt.float16
    act = mybir.ActivationFunctionType
    alu = mybir.AluOpType

    nc = bacc.Bacc("TRN2", target_bir_lowering=False, debug=False)
    xts = [nc.dram_tensor(f"xt{t}", [128, NK, bt], f16,
                          kind="ExternalInput").ap()
           for t, bt in enumerate(TILES)]
    wt = nc.dram_tensor("wt", [128, NK, 128], f16, kind="ExternalInput").ap()
    bb = nc.dram_tensor("bb", [128, 1], f32, kind="ExternalInput").ap()
    out = nc.dram_tensor("out", [BC, 1], f32, kind="ExternalOutput").ap()

    with tile.TileContext(nc) as tc:
        with (
            tc.tile_pool(name="consts", bufs=1) as consts,
            tc.tile_pool(name="xpool", bufs=8) as xpool,
            tc.tile_pool(name="ep", bufs=2) as ep,
            tc.tile_pool(name="small", bufs=3) as small,
            tc.tile_pool(name="psacc", bufs=2, space=bass.MemorySpace.PSUM) as psacc,
            tc.tile_pool(name="pstr", bufs=5, space=bass.MemorySpace.PSUM) as pstr,
        ):
            # ---- constants ----
            bb_sb = consts.tile([128, 1], f32)
            nc.scalar.dma_start(out=bb_sb, in_=bb)
            wt_sb = consts.tile([128, NK, 128], f16)
            k0 = 0
            for wc in W_CHUNKS:
                nc.scalar.dma_start(out=wt_sb[:, k0:k0 + wc, :],
                                    in_=wt[:, k0:k0 + wc, :])
                k0 += wc
            ident = consts.tile([128, 128], f32)
            make_identity(nc, ident)
            # load the (single) ACT function set during the DMA phase; Tanh
            # and Exp both live in "exp_and_others"
            warm = consts.tile([1, 1], f32)
            nc.vector.memset(warm, 0.0)
            nc.scalar.add(warm, warm, bb_sb[0:1, :])
            nc.scalar.activation(warm, warm, func=act.Tanh)
            nc.scalar.activation(warm, warm, func=act.Exp)

            fin16 = consts.tile([128, NOUT], f32)
            # out rows c*128+p == fin16[p, c]: iterate DRAM as [p][c] and a
            # plain DMA from [128, c] SBUF lands rows with no transpose
            outp = out.rearrange("(c p) o -> p (c o)", p=128)   # [128, NOUT]

            def emit_epilogue(t, acc):
                bt = TILES[t]
                nj = bt // 128
                col0 = sum(TILES[:t]) // 128
                accS = ep.tile([128, bt], f32, tag="accS")
                nc.scalar.add(accS, acc, bb_sb)
                zsum = small.tile([128, nj], f32, tag="zsum")
                s4h = small.tile([128, nj], f32, tag="s4h")
                for j in range(nj):
                    # own PSUM bank per 128-row group: ACT and DVE readers
                    # of different groups may then run concurrently
                    ps = pstr.tile([128, 128], f32, tag="psne",
                                   name=f"psne{t}_{j}")
                    nc.tensor.transpose(ps, accS[:, j * 128:(j + 1) * 128],
                                        ident)
                    v = ps[:, 0:64]
                    tv = small.tile([128, 8], f32, tag="tv")
                    nc.vector.max(tv, v)                  # top-8, descending
                    e_all = small.tile([128, 64], f32, tag="e_all")
                    nc.scalar.activation(e_all, v, func=act.Exp)
                    # exp of the 8th-largest: the top-8 mask threshold moves
                    # to exp-space (monotone), keeping the masking off PSUM
                    t8e = small.tile([128, 1], f32, tag="t8e")
                    nc.scalar.activation(t8e, tv[:, 7:8], func=act.Exp)
                    # expert half holds el/2, so tanh = 2*sigmoid(el)-1
                    th = small.tile([128, 64], f32, tag="th")
                    nc.scalar.activation(th, ps[:, 64:128], func=act.Tanh)
                    # g = exp(v) on the top-8 positions, exactly 0 elsewhere;
                    # zsum and the half-dot fall out of the fused accums
                    g = small.tile([128, 64], f32, tag="g")
                    nc.vector.scalar_tensor_tensor(
                        out=g, in0=e_all, scalar=t8e, in1=e_all,
                        op0=alu.is_ge, op1=alu.mult,
                        accum_out=zsum[:, j:j + 1])
                    scr = small.tile([128, 64], f32, tag="scr")
                    nc.vector.scalar_tensor_tensor(
                        out=scr, in0=g, scalar=0.5, in1=th,
                        op0=alu.mult, op1=alu.mult,
                        accum_out=s4h[:, j:j + 1])
                rz = small.tile([128, nj], f32, tag="rz")
                nc.vector.reciprocal(rz, zsum)
                # sigma = 0.5*tanh+0.5  =>  out = (0.5*s4)/zsum + 0.5
                if nj == 1:
                    nc.vector.tensor_scalar(
                        out=fin16[:, col0:col0 + 1], in0=s4h, scalar1=rz,
                        scalar2=0.5, op0=alu.mult, op1=alu.add)
                else:
                    fr = small.tile([128, nj], f32, tag="fr")
                    nc.vector.tensor_mul(fr, s4h, rz)
                    nc.vector.tensor_scalar(
                        out=fin16[:, col0:col0 + nj], in0=fr,
                        scalar1=0.5, scalar2=None, op0=alu.add)
                # Output DMAs: anything transferred before the x stream ends
                # delays the stream (one shared bandwidth device), so tiles
                # 0..NT-2 go out as ONE DMA whose read range spans all their
                # columns — it cannot fire until the second-to-last tile's
                # fin lands (~stream end), and the idle SP queue issues it
                # without touching the epilogue engines. The last tile's
                # single column rides Pool's SWDGE: cheapest fixed cost on
                # the final chain.
                if t == NT - 2:
                    nc.sync.dma_start(out=outp[:, 0:col0 + nj],
                                      in_=fin16[:, 0:col0 + nj])
                elif t == NT - 1:
                    nc.gpsimd.dma_start(out=outp[:, col0:col0 + nj],
                                        in_=fin16[:, col0:col0 + nj])

            pending = None
            for t in range(NT):
                bt = TILES[t]
                acc = psacc.tile([128, bt], f32, tag="acc", name=f"acc{t}")
                k = 0
                for c, ck in enumerate(CHUNKS[t]):
                    xk = xpool.tile([128, 16 * 512], f16, tag="xk")
                    xkv = xk.rearrange("p (a b) -> p a b", a=16 * 512 // bt,
                                       b=bt)
                    nc.sync.dma_start(out=xkv[:, 0:ck, :],
                                      in_=xts[t][:, k:k + ck, :])
                    for kc in range(ck):
                        nc.tensor.matmul(acc, lhsT=wt_sb[:, k + kc, :],
                                         rhs=xkv[:, kc, :],
                                         start=(k + kc == 0),
                                         stop=(k + kc == NK - 1))
                    k += ck
                    if c == 0 and pending is not None:
                        emit_epilogue(*pending)
                        pending = None
                pending = (t, acc)
            emit_epilogue(*pending)

    nc.compile()
    return nc


def get_program():
    if "prog" not in _cached:
        _cached["prog"] = _build_program()
    return _cached["prog"]


def make_in_maps(x, noise_w, noise_b, expert_w, expert_b):
    """Host-side sharding: per-core fp16 x slices + replicated fp16 weights.

    The expert weights/bias are halved so the on-chip tanh of the raw
    accumulator equals 2*sigmoid(expert_logit)-1.
    """
    w_comb = np.concatenate([noise_w, 0.5 * np.asarray(expert_w)],
                            axis=0).astype(np.float32)
    wt16 = w_comb.T.astype(np.float16)                       # [D, 128]
    wt = np.ascontiguousarray(
        wt16.reshape(NK, 128, 128).transpose(1, 0, 2))       # [128, NK, 128]
    bb = np.concatenate([noise_b, 0.5 * np.asarray(expert_b)]).astype(
        np.float32).reshape(128, 1)
    in_maps = []
    for c in range(NCORES):
        xs = x[c * BC:(c + 1) * BC, :].astype(np.float16)    # [BC, D]
        xsT = np.ascontiguousarray(xs.T)                     # [D, BC]
        im = {"wt": wt, "bb": bb}
        b0 = 0
        for t, bt in enumerate(TILES):
            # [p, nk, b]: contiguous per-partition blocks per tile
            im[f"xt{t}"] = np.ascontiguousarray(
                xsT[:, b0:b0 + bt].reshape(NK, 128, bt).transpose(1, 0, 2))
            b0 += bt
        in_maps.append(im)
    return in_maps


def kernel(x, noise, router_w, router_b, noise_w, noise_b, expert_w, expert_b,
           _trace=False):
    from concourse.bass_utils import run_bass_kernel_spmd

    x = np.asarray(x, dtype=np.float32)
    nc = get_program()
    in_maps = make_in_maps(x, np.asarray(noise_w), np.asarray(noise_b),
                           np.asarray(expert_w), np.asarray(expert_b))
    res = run_bass_kernel_spmd(nc, in_maps, core_ids=list(range(NCORES)),
                               trace=_trace)
    out = np.concatenate([r["out"] for r in res.results], axis=0)
    if _trace:
        kernel.last_results = res
    return out


# revision 22
# speedup vs baseline: 2.3409x; 1.0014x over previous
"""MoE logistic regression kernel for 8 Trainium2 NeuronCores.

Math (after dead-code elimination of the reference's unused router path):
    noise_logits = x @ noise_w.T + noise_b            # [B, E]
    top8 = top_k(noise_logits, 8)
    gates = softmax over the top-8 entries (others 0)
    expert = sigmoid(x @ expert_w.T + expert_b)       # [B, E]
    out[b] = sum_e gates[b,e] * expert[b,e]           # [B, 1]

Sharding: batch split 8 ways (2048 rows/core); weights replicated.

Implementation notes:
- Single-pass fp16 matmul (x and w rounded to fp16 on the host; fp32
  PSUM accumulate). Logit error ~4e-4 flips the top-8 set on only ~25
  of 16384 rows whose 8th/9th margin is that small; output l2 rel err
  ~1.2e-3, far under the 2e-2 gate. Halves DMA traffic and cuts PE
  work 3x vs an exact hi/lo split.
- noise_w/expert_w concatenated into one 128-wide stationary operand so
  x streams through the PE once for both matmuls.
- sigmoid(x) computed as 0.5*tanh(x/2)+0.5 with expert weights/bias
  pre-halved on the host: tanh and exp share one ACT function-table set
  so the kernel needs a single LoadActFuncSet, not 2x1283ns per tile.
- Batch tiles [512,512,512,256,128,128]: the taper keeps every
  epilogue except the last inside the DMA stream's shadow, and the
  final 128-row tile makes the last serial chain short. Tile t's
  epilogue instructions are emitted after tile t+1's first matmul
  chunk so the in-order PE queue never stalls on epilogue deps.
- Each transposed 128-row group gets its OWN PSUM bank: ScalarE and
  VectorE may only touch the same PSUM bank serially, so per-bank
  tiles let exp/tanh (ACT) run concurrently with Max8 (DVE) across
  groups. Top-8 selection is Max8 plus an is_ge mask applied in
  exp-space (monotone, so thresholding exp(v) against exp(t8) is the
  same selection but keeps the masking off PSUM); gates and the
  gate*expert dot are two fused scalar_tensor_tensor ops whose
  accum_out gives zsum and 0.5*s4 for free.
- Output needs no transpose: out rows c*128+p equal fin16[p, c], and a
  DRAM access pattern rearranged to [p][c] iterates in the same order
  as the SBUF source, so a strided DMA lands rows directly.
- x is staged host-side as [tile, partition, kchunk, col] fp16 so every
  DMA reads contiguous per-partition blocks (full 360GB/s); the last
  tile's trailing chunks shrink so the final matmuls start sooner.
"""

import sys

import numpy as np

if "/opt/trn_rl_repo" not in sys.path:
    sys.path.insert(0, "/opt/trn_rl_repo")

B, D, E, TOPK, NCORES = 16384, 4096, 64, 8, 8
BC = B // NCORES      # batch rows per core
NK = D // 128         # contraction chunks

TILES = [512, 512, 512, 256, 128, 128]   # batch rows per tile (sum = BC)
NT = len(TILES)
NOUT = BC // 128                         # output columns of fin16

# k-chunks per x DMA, per tile (sum = NK per tile)
CHUNKS = [
    [8, 8, 8, 8],
    [8, 8, 8, 8],
    [8, 8, 8, 8],
    [16, 16],
    [16, 16],
    [16, 8, 4, 2, 2],
]
W_CHUNKS = [8, 8, 8, 8]

_cached = {}


def _build_program():
    import concourse.bass as bass
    import concourse.tile as tile
    from concourse import bacc, mybir
    from concourse.masks import make_identity

    f32 = mybir.dt.float32
    f16 = mybir.dt.float16
    act = mybir.ActivationFunctionType
    alu = mybir.AluOpType

    nc = bacc.Bacc("TRN2", target_bir_lowering=False, debug=False)
    xts = [nc.dram_tensor(f"xt{t}", [128, NK, bt], f16,
                          kind="ExternalInput").ap()
           for t, bt in enumerate(TILES)]
    wt = nc.dram_tensor("wt", [128, NK, 128], f16, kind="ExternalInput").ap()
    bb = nc.dram_tensor("bb", [128, 1], f32, kind="ExternalInput").ap()
    out = nc.dram_tensor("out", [BC, 1], f32, kind="ExternalOutput").ap()

    with tile.TileContext(nc) as tc:
        with (
            tc.tile_pool(name="consts", bufs=1) as consts,
            tc.tile_pool(name="xpool", bufs=8) as xpool,
            tc.tile_pool(name="ep", bufs=2) as ep,
            tc.tile_pool(name="small", bufs=3) as small,
            tc.tile_pool(name="psacc", bufs=2, space=bass.MemorySpace.PSUM) as psacc,
            tc.tile_pool(name="pstr", bufs=5, space=bass.MemorySpace.PSUM) as pstr,
        ):
            # ---- constants ----
            bb_sb = consts.tile([128, 1], f32)
            nc.scalar.dma_start(out=bb_sb, in_=bb)
            wt_sb = consts.tile([128, NK, 128], f16)
            k0 = 0
            for wc in W_CHUNKS:
                nc.scalar.dma_start(out=wt_sb[:, k0:k0 + wc, :],
                                    in_=wt[:, k0:k0 + wc, :])
                k0 += wc
            ident = consts.tile([128, 128], f32)
            make_identity(nc, ident)
            # load the (single) ACT function set during the DMA phase; Tanh
            # and Exp both live in "exp_and_others"
            warm = consts.tile([1, 1], f32)
            nc.vector.memset(warm, 0.0)
            nc.scalar.add(warm, warm, bb_sb[0:1, :])
            nc.scalar.activation(warm, warm, func=act.Tanh)
            nc.scalar.activation(warm, warm, func=act.Exp)

            fin16 = consts.tile([128, NOUT], f32)
            # out rows c*128+p == fin16[p, c]: iterate DRAM as [p][c] and a
            # plain DMA from [128, c] SBUF lands rows with no transpose
            outp = out.rearrange("(c p) o -> p (c o)", p=128)   # [128, NOUT]

            def emit_epilogue(t, acc):
                bt = TILES[t]
                nj = bt // 128
                col0 = sum(TILES[:t]) // 128
                accS = ep.tile([128, bt], f32, tag="accS")
                if t == NT - 1:
                    # DVE add: off the ACT queue, which is still draining the
                    # previous tile's exps when the final accumulator lands
                    nc.vector.tensor_scalar_add(accS, acc, bb_sb)
                else:
                    nc.scalar.add(accS, acc, bb_sb)
                zsum = small.tile([128, nj], f32, tag="zsum")
                s4h = small.tile([128, nj], f32, tag="s4h")
                for j in range(nj):
                    # own PSUM bank per 128-row group: ACT and DVE readers
                    # of different groups may then run concurrently
                    ps = pstr.tile([128, 128], f32, tag="psne",
                                   name=f"psne{t}_{j}")
                    nc.tensor.transpose(ps, accS[:, j * 128:(j + 1) * 128],
                                        ident)
                    v = ps[:, 0:64]
                    tv = small.tile([128, 8], f32, tag="tv")
                    nc.vector.max(tv, v)                  # top-8, descending
                    e_all = small.tile([128, 64], f32, tag="e_all")
                    nc.scalar.activation(e_all, v, func=act.Exp)
                    # exp of the 8th-largest: the top-8 mask threshold moves
                    # to exp-space (monotone), keeping the masking off PSUM
                    t8e = small.tile([128, 1], f32, tag="t8e")
                    nc.scalar.activation(t8e, tv[:, 7:8], func=act.Exp)
                    # expert half holds el/2, so tanh = 2*sigmoid(el)-1
                    th = small.tile([128, 64], f32, tag="th")
                    nc.scalar.activation(th, ps[:, 64:128], func=act.Tanh)
                    # g = exp(v) on the top-8 positions, exactly 0 elsewhere;
                    # zsum and the half-dot fall out of the fused accums
                    g = small.tile([128, 64], f32, tag="g")
                    nc.vector.scalar_tensor_tensor(
                        out=g, in0=e_all, scalar=t8e, in1=e_all,
                        op0=alu.is_ge, op1=alu.mult,
                        accum_out=zsum[:, j:j + 1])
                    scr = small.tile([128, 64], f32, tag="scr")
                    nc.vector.scalar_tensor_tensor(
                        out=scr, in0=g, scalar=0.5, in1=th,
                        op0=alu.mult, op1=alu.mult,
                        accum_out=s4h[:, j:j + 1])
                rz = small.tile([128, nj], f32, tag="rz")
                nc.vector.reciprocal(rz, zsum)
                # sigma = 0.5*tanh+0.5  =>  out = (0.5*s4)/zsum + 0.5
                if nj == 1:
                    nc.vector.tensor_scalar(
                        out=fin16[:, col0:col0 + 1], in0=s4h, scalar1=rz,
                        scalar2=0.5, op0=alu.mult, op1=alu.add)
                else:
                    fr = small.tile([128, nj], f32, tag="fr")
                    nc.vector.tensor_mul(fr, s4h, rz)
                    nc.vector.tensor_scalar(
                        out=fin16[:, col0:col0 + nj], in0=fr,
                        scalar1=0.5, scalar2=None, op0=alu.add)
                # Output DMAs: anything transferred before the x stream ends
                # delays the stream (one shared bandwidth device), so tiles
                # 0..NT-2 go out as ONE DMA whose read range spans all their
                # columns — it cannot fire until the second-to-last tile's
                # fin lands (~stream end), and the idle SP queue issues it
                # without touching the epilogue engines. The last tile's
                # single column rides Pool's SWDGE: cheapest fixed cost on
                # the final chain.
                if t == NT - 2:
                    nc.sync.dma_start(out=outp[:, 0:col0 + nj],
                                      in_=fin16[:, 0:col0 + nj])
                elif t == NT - 1:
                    nc.gpsimd.dma_start(out=outp[:, col0:col0 + nj],
                                        in_=fin16[:, col0:col0 + nj])

            pending = None
            for t in range(NT):
                bt = TILES[t]
                acc = psacc.tile([128, bt], f32, tag="acc", name=f"acc{t}")
                k = 0
                for c, ck in enumerate(CHUNKS[t]):
                    xk = xpool.tile([128, 16 * 512], f16, tag="xk")
                    xkv = xk.rearrange("p (a b) -> p a b", a=16 * 512 // bt,
                                       b=bt)
                    nc.sync.dma_start(out=xkv[:, 0:ck, :],
                                      in_=xts[t][:, k:k + ck, :])
                    for kc in range(ck):
                        nc.tensor.matmul(acc, lhsT=wt_sb[:, k + kc, :],
                                         rhs=xkv[:, kc, :],
                                         start=(k + kc == 0),
                                         stop=(k + kc == NK - 1))
                    k += ck
                    if c == 0 and pending is not None:
                        emit_epilogue(*pending)
                        pending = None
                pending = (t, acc)
            emit_epilogue(*pending)

    nc.compile()
    return nc


def get_program():
    if "prog" not in _cached:
        _cached["prog"] = _build_program()
    return _cached["prog"]


def make_in_maps(x, noise_w, noise_b, expert_w, expert_b):
    """Host-side sharding: per-core fp16 x slices + replicated fp16 weights.

    The expert weights/bias are halved so the on-chip tanh of the raw
    accumulator equals 2*sigmoid(expert_logit)-1.
    """
    w_comb = np.concatenate([noise_w, 0.5 * np.asarray(expert_w)],
                            axis=0).astype(np.float32)
    wt16 = w_comb.T.astype(np.float16)                       # [D, 128]
    wt = np.ascontiguousarray(
        wt16.reshape(NK, 128, 128).transpose(1, 0, 2))       # [128, NK, 128]
    bb = np.concatenate([noise_b, 0.5 * np.asarray(expert_b)]).astype(
        np.float32).reshape(128, 1)
    in_maps = []
    for c in range(NCORES):
        xs = x[c * BC:(c + 1) * BC, :].astype(np.float16)    # [BC, D]
        xsT = np.ascontiguousarray(xs.T)                     # [D, BC]
        im = {"wt": wt, "bb": bb}
        b0 = 0
        for t, bt in enumerate(TILES):
            # [p, nk, b]: contiguous per-partition blocks per tile
            im[f"xt{t}"] = np.ascontiguousarray(
                xsT[:, b0:b0 + bt].reshape(NK, 128, bt).transpose(1, 0, 2))
            b0 += bt
        in_maps.append(im)
    return in_maps


def kernel(x, noise, router_w, router_b, noise_w, noise_b, expert_w, expert_b,
           _trace=False):
    from concourse.bass_utils import run_bass_kernel_spmd

    x = np.asarray(x, dtype=np.float32)
    nc = get_program()
    in_maps = make_in_maps(x, np.asarray(noise_w), np.asarray(noise_b),
                           np.asarray(expert_w), np.asarray(expert_b))
    res = run_bass_kernel_spmd(nc, in_maps, core_ids=list(range(NCORES)),
                               trace=_trace)
    out = np.concatenate([r["out"] for r in res.results], axis=0)
    if _trace:
        kernel.last_results = res
    return out


# revision 23
# speedup vs baseline: 2.3532x; 1.0053x over previous
"""MoE logistic regression kernel for 8 Trainium2 NeuronCores.

Math (after dead-code elimination of the reference's unused router path):
    noise_logits = x @ noise_w.T + noise_b            # [B, E]
    top8 = top_k(noise_logits, 8)
    gates = softmax over the top-8 entries (others 0)
    expert = sigmoid(x @ expert_w.T + expert_b)       # [B, E]
    out[b] = sum_e gates[b,e] * expert[b,e]           # [B, 1]

Sharding: batch split 8 ways (2048 rows/core); weights replicated.

Implementation notes:
- Single-pass fp16 matmul (x and w rounded to fp16 on the host; fp32
  PSUM accumulate). Logit error ~4e-4 flips the top-8 set on only ~25
  of 16384 rows whose 8th/9th margin is that small; output l2 rel err
  ~1.2e-3, far under the 2e-2 gate. Halves DMA traffic and cuts PE
  work 3x vs an exact hi/lo split.
- noise_w/expert_w concatenated into one 128-wide stationary operand so
  x streams through the PE once for both matmuls.
- sigmoid(x) computed as 0.5*tanh(x/2)+0.5 with expert weights/bias
  pre-halved on the host: tanh and exp share one ACT function-table set
  so the kernel needs a single LoadActFuncSet, not 2x1283ns per tile.
- Batch tiles [512,512,512,256,128,128]: the taper keeps every
  epilogue except the last inside the DMA stream's shadow, and the
  final 128-row tile makes the last serial chain short. Tile t's
  epilogue instructions are emitted after tile t+1's first matmul
  chunk so the in-order PE queue never stalls on epilogue deps.
- Each transposed 128-row group gets its OWN PSUM bank: ScalarE and
  VectorE may only touch the same PSUM bank serially, so per-bank
  tiles let exp/tanh (ACT) run concurrently with Max8 (DVE) across
  groups. Top-8 selection is Max8 plus an is_ge mask applied in
  exp-space (monotone, so thresholding exp(v) against exp(t8) is the
  same selection but keeps the masking off PSUM); gates and the
  gate*expert dot are two fused scalar_tensor_tensor ops whose
  accum_out gives zsum and 0.5*s4 for free.
- Output needs no transpose: out rows c*128+p equal fin16[p, c], and a
  DRAM access pattern rearranged to [p][c] iterates in the same order
  as the SBUF source, so a strided DMA lands rows directly.
- x is staged host-side as [tile, partition, kchunk, col] fp16 so every
  DMA reads contiguous per-partition blocks (full 360GB/s); the last
  tile's trailing chunks shrink so the final matmuls start sooner.
"""

import sys

import numpy as np

if "/opt/trn_rl_repo" not in sys.path:
    sys.path.insert(0, "/opt/trn_rl_repo")

B, D, E, TOPK, NCORES = 16384, 4096, 64, 8, 8
BC = B // NCORES      # batch rows per core
NK = D // 128         # contraction chunks

TILES = [512, 512, 512, 256, 128, 128]   # batch rows per tile (sum = BC)
NT = len(TILES)
NOUT = BC // 128                         # output columns of fin16

# k-chunks per x DMA, per tile (sum = NK per tile)
CHUNKS = [
    [8, 8, 8, 8],
    [8, 8, 8, 8],
    [8, 8, 8, 8],
    [16, 16],
    [16, 16],
    [8, 8, 8, 4, 4],
]
W_CHUNKS = [8, 8, 8, 8]

_cached = {}


def _build_program():
    import concourse.bass as bass
    import concourse.tile as tile
    from concourse import bacc, mybir
    from concourse.masks import make_identity

    f32 = mybir.dt.float32
    f16 = mybir.dt.float16
    act = mybir.ActivationFunctionType
    alu = mybir.AluOpType

    nc = bacc.Bacc("TRN2", target_bir_lowering=False, debug=False)
    xts = [nc.dram_tensor(f"xt{t}", [128, NK, bt], f16,
                          kind="ExternalInput").ap()
           for t, bt in enumerate(TILES)]
    wt = nc.dram_tensor("wt", [128, NK, 128], f16, kind="ExternalInput").ap()
    bb = nc.dram_tensor("bb", [128, 1], f32, kind="ExternalInput").ap()
    out = nc.dram_tensor("out", [BC, 1], f32, kind="ExternalOutput").ap()

    with tile.TileContext(nc) as tc:
        with (
            tc.tile_pool(name="consts", bufs=1) as consts,
            tc.tile_pool(name="xpool", bufs=8) as xpool,
            tc.tile_pool(name="ep", bufs=2) as ep,
            tc.tile_pool(name="small", bufs=3) as small,
            tc.tile_pool(name="psacc", bufs=2, space=bass.MemorySpace.PSUM) as psacc,
            tc.tile_pool(name="pstr", bufs=5, space=bass.MemorySpace.PSUM) as pstr,
        ):
            # ---- constants ----
            bb_sb = consts.tile([128, 1], f32)
            nc.scalar.dma_start(out=bb_sb, in_=bb)
            wt_sb = consts.tile([128, NK, 128], f16)
            k0 = 0
            for wc in W_CHUNKS:
                nc.scalar.dma_start(out=wt_sb[:, k0:k0 + wc, :],
                                    in_=wt[:, k0:k0 + wc, :])
                k0 += wc
            ident = consts.tile([128, 128], f32)
            make_identity(nc, ident)
            # load the (single) ACT function set during the DMA phase; Tanh
            # and Exp both live in "exp_and_others"
            warm = consts.tile([1, 1], f32)
            nc.vector.memset(warm, 0.0)
            nc.scalar.add(warm, warm, bb_sb[0:1, :])
            nc.scalar.activation(warm, warm, func=act.Tanh)
            nc.scalar.activation(warm, warm, func=act.Exp)

            fin16 = consts.tile([128, NOUT], f32)
            # out rows c*128+p == fin16[p, c]: iterate DRAM as [p][c] and a
            # plain DMA from [128, c] SBUF lands rows with no transpose
            outp = out.rearrange("(c p) o -> p (c o)", p=128)   # [128, NOUT]

            def emit_epilogue(t, acc):
                bt = TILES[t]
                nj = bt // 128
                col0 = sum(TILES[:t]) // 128
                accS = ep.tile([128, bt], f32, tag="accS")
                if t == NT - 1:
                    # DVE add: off the ACT queue, which is still draining the
                    # previous tile's exps when the final accumulator lands
                    nc.vector.tensor_scalar_add(accS, acc, bb_sb)
                else:
                    nc.scalar.add(accS, acc, bb_sb)
                zsum = small.tile([128, nj], f32, tag="zsum")
                s4h = small.tile([128, nj], f32, tag="s4h")
                for j in range(nj):
                    # own PSUM bank per 128-row group: ACT and DVE readers
                    # of different groups may then run concurrently
                    ps = pstr.tile([128, 128], f32, tag="psne",
                                   name=f"psne{t}_{j}")
                    nc.tensor.transpose(ps, accS[:, j * 128:(j + 1) * 128],
                                        ident)
                    v = ps[:, 0:64]
                    tv = small.tile([128, 8], f32, tag="tv")
                    nc.vector.max(tv, v)                  # top-8, descending
                    e_all = small.tile([128, 64], f32, tag="e_all")
                    nc.scalar.activation(e_all, v, func=act.Exp)
                    # exp of the 8th-largest: the top-8 mask threshold moves
                    # to exp-space (monotone), keeping the masking off PSUM
                    t8e = small.tile([128, 1], f32, tag="t8e")
                    nc.scalar.activation(t8e, tv[:, 7:8], func=act.Exp)
                    # expert half holds el/2, so tanh = 2*sigmoid(el)-1
                    th = small.tile([128, 64], f32, tag="th")
                    nc.scalar.activation(th, ps[:, 64:128], func=act.Tanh)
                    # g = exp(v) on the top-8 positions, exactly 0 elsewhere;
                    # zsum and the half-dot fall out of the fused accums
                    g = small.tile([128, 64], f32, tag="g")
                    nc.vector.scalar_tensor_tensor(
                        out=g, in0=e_all, scalar=t8e, in1=e_all,
                        op0=alu.is_ge, op1=alu.mult,
                        accum_out=zsum[:, j:j + 1])
                    scr = small.tile([128, 64], f32, tag="scr")
                    nc.vector.scalar_tensor_tensor(
                        out=scr, in0=g, scalar=0.5, in1=th,
                        op0=alu.mult, op1=alu.mult,
                        accum_out=s4h[:, j:j + 1])
                rz = small.tile([128, nj], f32, tag="rz")
                nc.vector.reciprocal(rz, zsum)
                # sigma = 0.5*tanh+0.5  =>  out = (0.5*s4)/zsum + 0.5
                if nj == 1:
                    nc.vector.tensor_scalar(
                        out=fin16[:, col0:col0 + 1], in0=s4h, scalar1=rz,
                        scalar2=0.5, op0=alu.mult, op1=alu.add)
                else:
                    fr = small.tile([128, nj], f32, tag="fr")
                    nc.vector.tensor_mul(fr, s4h, rz)
                    nc.vector.tensor_scalar(
                        out=fin16[:, col0:col0 + nj], in0=fr,
                        scalar1=0.5, scalar2=None, op0=alu.add)
                # Output DMAs: anything transferred before the x stream ends
                # delays the stream (one shared bandwidth device), so tiles
                # 0..NT-2 go out as ONE DMA whose read range spans all their
                # columns — it cannot fire until the second-to-last tile's
                # fin lands (~stream end), and the idle SP queue issues it
                # without touching the epilogue engines. The last tile's
                # single column rides Pool's SWDGE: cheapest fixed cost on
                # the final chain.
                if t == NT - 2:
                    nc.sync.dma_start(out=outp[:, 0:col0 + nj],
                                      in_=fin16[:, 0:col0 + nj])
                elif t == NT - 1:
                    nc.gpsimd.dma_start(out=outp[:, col0:col0 + nj],
                                        in_=fin16[:, col0:col0 + nj])

            pending = None
            for t in range(NT):
                bt = TILES[t]
                acc = psacc.tile([128, bt], f32, tag="acc", name=f"acc{t}")
                k = 0
                for c, ck in enumerate(CHUNKS[t]):
                    xk = xpool.tile([128, 16 * 512], f16, tag="xk")
                    xkv = xk.rearrange("p (a b) -> p a b", a=16 * 512 // bt,
                                       b=bt)
                    nc.sync.dma_start(out=xkv[:, 0:ck, :],
                                      in_=xts[t][:, k:k + ck, :])
                    for kc in range(ck):
                        nc.tensor.matmul(acc, lhsT=wt_sb[:, k + kc, :],
                                         rhs=xkv[:, kc, :],
                                         start=(k + kc == 0),
                                         stop=(k + kc == NK - 1))
                    k += ck
                    if c == 0 and pending is not None:
                        emit_epilogue(*pending)
                        pending = None
                pending = (t, acc)
            emit_epilogue(*pending)

    nc.compile()
    return nc


def get_program():
    if "prog" not in _cached:
        _cached["prog"] = _build_program()
    return _cached["prog"]


def make_in_maps(x, noise_w, noise_b, expert_w, expert_b):
    """Host-side sharding: per-core fp16 x slices + replicated fp16 weights.

    The expert weights/bias are halved so the on-chip tanh of the raw
    accumulator equals 2*sigmoid(expert_logit)-1.
    """
    w_comb = np.concatenate([noise_w, 0.5 * np.asarray(expert_w)],
                            axis=0).astype(np.float32)
    wt16 = w_comb.T.astype(np.float16)                       # [D, 128]
    wt = np.ascontiguousarray(
        wt16.reshape(NK, 128, 128).transpose(1, 0, 2))       # [128, NK, 128]
    bb = np.concatenate([noise_b, 0.5 * np.asarray(expert_b)]).astype(
        np.float32).reshape(128, 1)
    in_maps = []
    for c in range(NCORES):
        xs = x[c * BC:(c + 1) * BC, :].astype(np.float16)    # [BC, D]
        xsT = np.ascontiguousarray(xs.T)                     # [D, BC]
        im = {"wt": wt, "bb": bb}
        b0 = 0
        for t, bt in enumerate(TILES):
            # [p, nk, b]: contiguous per-partition blocks per tile
            im[f"xt{t}"] = np.ascontiguousarray(
                xsT[:, b0:b0 + bt].reshape(NK, 128, bt).transpose(1, 0, 2))
            b0 += bt
        in_maps.append(im)
    return in_maps


def kernel(x, noise, router_w, router_b, noise_w, noise_b, expert_w, expert_b,
           _trace=False):
    from concourse.bass_utils import run_bass_kernel_spmd

    x = np.asarray(x, dtype=np.float32)
    nc = get_program()
    in_maps = make_in_maps(x, np.asarray(noise_w), np.asarray(noise_b),
                           np.asarray(expert_w), np.asarray(expert_b))
    res = run_bass_kernel_spmd(nc, in_maps, core_ids=list(range(NCORES)),
                               trace=_trace)
    out = np.concatenate([r["out"] for r in res.results], axis=0)
    if _trace:
        kernel.last_results = res
    return out


# revision 24
# speedup vs baseline: 2.3591x; 1.0025x over previous
"""MoE logistic regression kernel for 8 Trainium2 NeuronCores.

Math (after dead-code elimination of the reference's unused router path):
    noise_logits = x @ noise_w.T + noise_b            # [B, E]
    top8 = top_k(noise_logits, 8)
    gates = softmax over the top-8 entries (others 0)
    expert = sigmoid(x @ expert_w.T + expert_b)       # [B, E]
    out[b] = sum_e gates[b,e] * expert[b,e]           # [B, 1]

Sharding: batch split 8 ways (2048 rows/core); weights replicated.

Implementation notes:
- Single-pass fp16 matmul (x and w rounded to fp16 on the host; fp32
  PSUM accumulate). Logit error ~4e-4 flips the top-8 set on only ~25
  of 16384 rows whose 8th/9th margin is that small; output l2 rel err
  ~1.2e-3, far under the 2e-2 gate. Halves DMA traffic and cuts PE
  work 3x vs an exact hi/lo split.
- noise_w/expert_w concatenated into one 128-wide stationary operand so
  x streams through the PE once for both matmuls.
- sigmoid(x) computed as 0.5*tanh(x/2)+0.5 with expert weights/bias
  pre-halved on the host: tanh and exp share one ACT function-table set
  so the kernel needs a single LoadActFuncSet, not 2x1283ns per tile.
- Batch tiles [512,512,512,256,128,128]: the taper keeps every
  epilogue except the last inside the DMA stream's shadow, and the
  final 128-row tile makes the last serial chain short. Tile t's
  epilogue instructions are emitted after tile t+1's first matmul
  chunk so the in-order PE queue never stalls on epilogue deps.
- Each transposed 128-row group gets its OWN PSUM bank: ScalarE and
  VectorE may only touch the same PSUM bank serially, so per-bank
  tiles let exp/tanh (ACT) run concurrently with Max8 (DVE) across
  groups. Top-8 selection is Max8 plus an is_ge mask applied in
  exp-space (monotone, so thresholding exp(v) against exp(t8) is the
  same selection but keeps the masking off PSUM); gates and the
  gate*expert dot are two fused scalar_tensor_tensor ops whose
  accum_out gives zsum and 0.5*s4 for free.
- Output needs no transpose: out rows c*128+p equal fin16[p, c], and a
  DRAM access pattern rearranged to [p][c] iterates in the same order
  as the SBUF source, so a strided DMA lands rows directly.
- x is staged host-side as [tile, partition, kchunk, col] fp16 so every
  DMA reads contiguous per-partition blocks (full 360GB/s); the last
  tile's trailing chunks shrink so the final matmuls start sooner.
"""

import sys

import numpy as np

if "/opt/trn_rl_repo" not in sys.path:
    sys.path.insert(0, "/opt/trn_rl_repo")

B, D, E, TOPK, NCORES = 16384, 4096, 64, 8, 8
BC = B // NCORES      # batch rows per core
NK = D // 128         # contraction chunks

TILES = [512, 512, 512, 256, 128, 128]   # batch rows per tile (sum = BC)
NT = len(TILES)
NOUT = BC // 128                         # output columns of fin16

# k-chunks per x DMA, per tile (sum = NK per tile)
CHUNKS = [
    [8, 8, 8, 8],
    [8, 8, 8, 8],
    [8, 8, 8, 8],
    [8, 8, 8, 8],
    [8, 8, 8, 8],
    [8, 8, 8, 4, 4],
]
W_CHUNKS = [8, 8, 8, 8]

_cached = {}


def _build_program():
    import concourse.bass as bass
    import concourse.tile as tile
    from concourse import bacc, mybir
    from concourse.masks import make_identity

    f32 = mybir.dt.float32
    f16 = mybir.dt.float16
    act = mybir.ActivationFunctionType
    alu = mybir.AluOpType

    nc = bacc.Bacc("TRN2", target_bir_lowering=False, debug=False)
    xts = [nc.dram_tensor(f"xt{t}", [128, NK, bt], f16,
                          kind="ExternalInput").ap()
           for t, bt in enumerate(TILES)]
    wt = nc.dram_tensor("wt", [128, NK, 128], f16, kind="ExternalInput").ap()
    bb = nc.dram_tensor("bb", [128, 1], f32, kind="ExternalInput").ap()
    out = nc.dram_tensor("out", [BC, 1], f32, kind="ExternalOutput").ap()

    with tile.TileContext(nc) as tc:
        with (
            tc.tile_pool(name="consts", bufs=1) as consts,
            tc.tile_pool(name="xpool", bufs=8) as xpool,
            tc.tile_pool(name="ep", bufs=2) as ep,
            tc.tile_pool(name="small", bufs=3) as small,
            tc.tile_pool(name="psacc", bufs=2, space=bass.MemorySpace.PSUM) as psacc,
            tc.tile_pool(name="pstr", bufs=5, space=bass.MemorySpace.PSUM) as pstr,
        ):
            # ---- constants ----
            bb_sb = consts.tile([128, 1], f32)
            nc.scalar.dma_start(out=bb_sb, in_=bb)
            wt_sb = consts.tile([128, NK, 128], f16)
            k0 = 0
            for wc in W_CHUNKS:
                nc.scalar.dma_start(out=wt_sb[:, k0:k0 + wc, :],
                                    in_=wt[:, k0:k0 + wc, :])
                k0 += wc
            ident = consts.tile([128, 128], f32)
            make_identity(nc, ident)
            # load the (single) ACT function set during the DMA phase; Tanh
            # and Exp both live in "exp_and_others"
            warm = consts.tile([1, 1], f32)
            nc.vector.memset(warm, 0.0)
            nc.scalar.add(warm, warm, bb_sb[0:1, :])
            nc.scalar.activation(warm, warm, func=act.Tanh)
            nc.scalar.activation(warm, warm, func=act.Exp)

            fin16 = consts.tile([128, NOUT], f32)
            # out rows c*128+p == fin16[p, c]: iterate DRAM as [p][c] and a
            # plain DMA from [128, c] SBUF lands rows with no transpose
            outp = out.rearrange("(c p) o -> p (c o)", p=128)   # [128, NOUT]

            def emit_epilogue(t, acc):
                bt = TILES[t]
                nj = bt // 128
                col0 = sum(TILES[:t]) // 128
                accS = ep.tile([128, bt], f32, tag="accS")
                if t == NT - 1:
                    # DVE add: off the ACT queue, which is still draining the
                    # previous tile's exps when the final accumulator lands
                    nc.vector.tensor_scalar_add(accS, acc, bb_sb)
                else:
                    nc.scalar.add(accS, acc, bb_sb)
                zsum = small.tile([128, nj], f32, tag="zsum")
                s4h = small.tile([128, nj], f32, tag="s4h")
                for j in range(nj):
                    # own PSUM bank per 128-row group: ACT and DVE readers
                    # of different groups may then run concurrently
                    ps = pstr.tile([128, 128], f32, tag="psne",
                                   name=f"psne{t}_{j}")
                    nc.tensor.transpose(ps, accS[:, j * 128:(j + 1) * 128],
                                        ident)
                    v = ps[:, 0:64]
                    tv = small.tile([128, 8], f32, tag="tv")
                    nc.vector.max(tv, v)                  # top-8, descending
                    e_all = small.tile([128, 64], f32, tag="e_all")
                    nc.scalar.activation(e_all, v, func=act.Exp)
                    # exp of the 8th-largest: the top-8 mask threshold moves
                    # to exp-space (monotone), keeping the masking off PSUM
                    t8e = small.tile([128, 1], f32, tag="t8e")
                    nc.scalar.activation(t8e, tv[:, 7:8], func=act.Exp)
                    # expert half holds el/2, so tanh = 2*sigmoid(el)-1
                    th = small.tile([128, 64], f32, tag="th")
                    nc.scalar.activation(th, ps[:, 64:128], func=act.Tanh)
                    # g = exp(v) on the top-8 positions, exactly 0 elsewhere;
                    # zsum and the half-dot fall out of the fused accums
                    g = small.tile([128, 64], f32, tag="g")
                    nc.vector.scalar_tensor_tensor(
                        out=g, in0=e_all, scalar=t8e, in1=e_all,
                        op0=alu.is_ge, op1=alu.mult,
                        accum_out=zsum[:, j:j + 1])
                    scr = small.tile([128, 64], f32, tag="scr")
                    nc.vector.scalar_tensor_tensor(
                        out=scr, in0=g, scalar=0.5, in1=th,
                        op0=alu.mult, op1=alu.mult,
                        accum_out=s4h[:, j:j + 1])
                rz = small.tile([128, nj], f32, tag="rz")
                nc.vector.reciprocal(rz, zsum)
                # sigma = 0.5*tanh+0.5  =>  out = (0.5*s4)/zsum + 0.5
                if nj == 1:
                    nc.vector.tensor_scalar(
                        out=fin16[:, col0:col0 + 1], in0=s4h, scalar1=rz,
                        scalar2=0.5, op0=alu.mult, op1=alu.add)
                else:
                    fr = small.tile([128, nj], f32, tag="fr")
                    nc.vector.tensor_mul(fr, s4h, rz)
                    nc.vector.tensor_scalar(
                        out=fin16[:, col0:col0 + nj], in0=fr,
                        scalar1=0.5, scalar2=None, op0=alu.add)
                # Output DMAs: anything transferred before the x stream ends
                # delays the stream (one shared bandwidth device), so tiles
                # 0..NT-2 go out as ONE DMA whose read range spans all their
                # columns — it cannot fire until the second-to-last tile's
                # fin lands (~stream end), and the idle SP queue issues it
                # without touching the epilogue engines. The last tile's
                # single column rides Pool's SWDGE: cheapest fixed cost on
                # the final chain.
                if t == NT - 2:
                    nc.sync.dma_start(out=outp[:, 0:col0 + nj],
                                      in_=fin16[:, 0:col0 + nj])
                elif t == NT - 1:
                    nc.gpsimd.dma_start(out=outp[:, col0:col0 + nj],
                                        in_=fin16[:, col0:col0 + nj])

            pending = None
            for t in range(NT):
                bt = TILES[t]
                acc = psacc.tile([128, bt], f32, tag="acc", name=f"acc{t}")
                k = 0
                for c, ck in enumerate(CHUNKS[t]):
                    xk = xpool.tile([128, 16 * 512], f16, tag="xk")
                    xkv = xk.rearrange("p (a b) -> p a b", a=16 * 512 // bt,
                                       b=bt)
                    nc.sync.dma_start(out=xkv[:, 0:ck, :],
                                      in_=xts[t][:, k:k + ck, :])
                    for kc in range(ck):
                        nc.tensor.matmul(acc, lhsT=wt_sb[:, k + kc, :],
                                         rhs=xkv[:, kc, :],
                                         start=(k + kc == 0),
                                         stop=(k + kc == NK - 1))
                    k += ck
                    if c == 0 and pending is not None:
                        emit_epilogue(*pending)
                        pending = None
                pending = (t, acc)
            emit_epilogue(*pending)

    nc.compile()
    return nc


def get_program():
    if "prog" not in _cached:
        _cached["prog"] = _build_program()
    return _cached["prog"]


def make_in_maps(x, noise_w, noise_b, expert_w, expert_b):
    """Host-side sharding: per-core fp16 x slices + replicated fp16 weights.

    The expert weights/bias are halved so the on-chip tanh of the raw
    accumulator equals 2*sigmoid(expert_logit)-1.
    """
    w_comb = np.concatenate([noise_w, 0.5 * np.asarray(expert_w)],
                            axis=0).astype(np.float32)
    wt16 = w_comb.T.astype(np.float16)                       # [D, 128]
    wt = np.ascontiguousarray(
        wt16.reshape(NK, 128, 128).transpose(1, 0, 2))       # [128, NK, 128]
    bb = np.concatenate([noise_b, 0.5 * np.asarray(expert_b)]).astype(
        np.float32).reshape(128, 1)
    in_maps = []
    for c in range(NCORES):
        xs = x[c * BC:(c + 1) * BC, :].astype(np.float16)    # [BC, D]
        xsT = np.ascontiguousarray(xs.T)                     # [D, BC]
        im = {"wt": wt, "bb": bb}
        b0 = 0
        for t, bt in enumerate(TILES):
            # [p, nk, b]: contiguous per-partition blocks per tile
            im[f"xt{t}"] = np.ascontiguousarray(
                xsT[:, b0:b0 + bt].reshape(NK, 128, bt).transpose(1, 0, 2))
            b0 += bt
        in_maps.append(im)
    return in_maps


def kernel(x, noise, router_w, router_b, noise_w, noise_b, expert_w, expert_b,
           _trace=False):
    from concourse.bass_utils import run_bass_kernel_spmd

    x = np.asarray(x, dtype=np.float32)
    nc = get_program()
    in_maps = make_in_maps(x, np.asarray(noise_w), np.asarray(noise_b),
                           np.asarray(expert_w), np.asarray(expert_b))
    res = run_bass_kernel_spmd(nc, in_maps, core_ids=list(range(NCORES)),
                               trace=_trace)
    out = np.concatenate([r["out"] for r in res.results], axis=0)
    if _trace:
        kernel.last_results = res
    return out


# revision 25
# speedup vs baseline: 2.3656x; 1.0027x over previous
"""MoE logistic regression kernel for 8 Trainium2 NeuronCores.

Math (after dead-code elimination of the reference's unused router path):
    noise_logits = x @ noise_w.T + noise_b            # [B, E]
    top8 = top_k(noise_logits, 8)
    gates = softmax over the top-8 entries (others 0)
    expert = sigmoid(x @ expert_w.T + expert_b)       # [B, E]
    out[b] = sum_e gates[b,e] * expert[b,e]           # [B, 1]

Sharding: batch split 8 ways (2048 rows/core); weights replicated.

Implementation notes:
- Single-pass fp16 matmul (x and w rounded to fp16 on the host; fp32
  PSUM accumulate). Logit error ~4e-4 flips the top-8 set on only ~25
  of 16384 rows whose 8th/9th margin is that small; output l2 rel err
  ~1.2e-3, far under the 2e-2 gate. Halves DMA traffic and cuts PE
  work 3x vs an exact hi/lo split.
- noise_w/expert_w concatenated into one 128-wide stationary operand so
  x streams through the PE once for both matmuls.
- sigmoid(x) computed as 0.5*tanh(x/2)+0.5 with expert weights/bias
  pre-halved on the host: tanh and exp share one ACT function-table set
  so the kernel needs a single LoadActFuncSet, not 2x1283ns per tile.
- Batch tiles [512,512,512,256,128,128]: the taper keeps every
  epilogue except the last inside the DMA stream's shadow, and the
  final 128-row tile makes the last serial chain short. Tile t's
  epilogue instructions are emitted after tile t+1's first matmul
  chunk so the in-order PE queue never stalls on epilogue deps.
- Each transposed 128-row group gets its OWN PSUM bank: ScalarE and
  VectorE may only touch the same PSUM bank serially, so per-bank
  tiles let exp/tanh (ACT) run concurrently with Max8 (DVE) across
  groups. Top-8 selection is Max8 plus an is_ge mask applied in
  exp-space (monotone, so thresholding exp(v) against exp(t8) is the
  same selection but keeps the masking off PSUM); gates and the
  gate*expert dot are two fused scalar_tensor_tensor ops whose
  accum_out gives zsum and 0.5*s4 for free.
- Output needs no transpose: out rows c*128+p equal fin16[p, c], and a
  DRAM access pattern rearranged to [p][c] iterates in the same order
  as the SBUF source, so a strided DMA lands rows directly.
- x is staged host-side as [tile, partition, kchunk, col] fp16 so every
  DMA reads contiguous per-partition blocks (full 360GB/s); the last
  tile's trailing chunks shrink so the final matmuls start sooner.
"""

import sys

import numpy as np

if "/opt/trn_rl_repo" not in sys.path:
    sys.path.insert(0, "/opt/trn_rl_repo")

B, D, E, TOPK, NCORES = 16384, 4096, 64, 8, 8
BC = B // NCORES      # batch rows per core
NK = D // 128         # contraction chunks

TILES = [512, 512, 512, 256, 128, 128]   # batch rows per tile (sum = BC)
NT = len(TILES)
NOUT = BC // 128                         # output columns of fin16

# k-chunks per x DMA, per tile (sum = NK per tile)
CHUNKS = [
    [8, 8, 8, 8],
    [8, 8, 8, 8],
    [8, 8, 8, 8],
    [8, 8, 8, 8],
    [8, 8, 8, 8],
    [8, 8, 8, 4, 4],
]
W_CHUNKS = [8, 8, 8, 8]

_cached = {}


def _build_program():
    import concourse.bass as bass
    import concourse.tile as tile
    from concourse import bacc, mybir
    from concourse.masks import make_identity

    f32 = mybir.dt.float32
    f16 = mybir.dt.float16
    act = mybir.ActivationFunctionType
    alu = mybir.AluOpType

    nc = bacc.Bacc("TRN2", target_bir_lowering=False, debug=False)
    xts = [nc.dram_tensor(f"xt{t}", [128, NK, bt], f16,
                          kind="ExternalInput").ap()
           for t, bt in enumerate(TILES)]
    wt = nc.dram_tensor("wt", [128, NK, 128], f16, kind="ExternalInput").ap()
    bb = nc.dram_tensor("bb", [128, 1], f32, kind="ExternalInput").ap()
    out = nc.dram_tensor("out", [BC, 1], f32, kind="ExternalOutput").ap()

    with tile.TileContext(nc) as tc:
        with (
            tc.tile_pool(name="consts", bufs=1) as consts,
            tc.tile_pool(name="xpool", bufs=8) as xpool,
            tc.tile_pool(name="ep", bufs=2) as ep,
            tc.tile_pool(name="small", bufs=3) as small,
            tc.tile_pool(name="psacc", bufs=2, space=bass.MemorySpace.PSUM) as psacc,
            tc.tile_pool(name="pstr", bufs=5, space=bass.MemorySpace.PSUM) as pstr,
        ):
            # ---- constants ----
            bb_sb = consts.tile([128, 1], f32)
            nc.scalar.dma_start(out=bb_sb, in_=bb)
            wt_sb = consts.tile([128, NK, 128], f16)
            k0 = 0
            for wc in W_CHUNKS:
                nc.scalar.dma_start(out=wt_sb[:, k0:k0 + wc, :],
                                    in_=wt[:, k0:k0 + wc, :])
                k0 += wc
            ident = consts.tile([128, 128], f32)
            make_identity(nc, ident)
            # load the (single) ACT function set during the DMA phase; Tanh
            # and Exp both live in "exp_and_others"
            warm = consts.tile([1, 1], f32)
            nc.vector.memset(warm, 0.0)
            nc.scalar.add(warm, warm, bb_sb[0:1, :])
            nc.scalar.activation(warm, warm, func=act.Tanh)
            nc.scalar.activation(warm, warm, func=act.Exp)

            fin16 = consts.tile([128, NOUT], f32)
            # out rows c*128+p == fin16[p, c]: iterate DRAM as [p][c] and a
            # plain DMA from [128, c] SBUF lands rows with no transpose
            outp = out.rearrange("(c p) o -> p (c o)", p=128)   # [128, NOUT]

            def emit_epilogue(t, acc):
                bt = TILES[t]
                nj = bt // 128
                col0 = sum(TILES[:t]) // 128
                accS = ep.tile([128, bt], f32, tag="accS")
                if t == NT - 1:
                    # DVE add: off the ACT queue, which is still draining the
                    # previous tile's exps when the final accumulator lands
                    nc.vector.tensor_scalar_add(accS, acc, bb_sb)
                else:
                    nc.scalar.add(accS, acc, bb_sb)
                zsum = small.tile([128, nj], f32, tag="zsum")
                s4h = small.tile([128, nj], f32, tag="s4h")
                for j in range(nj):
                    # own PSUM bank per 128-row group: ACT and DVE readers
                    # of different groups may then run concurrently
                    ps = pstr.tile([128, 128], f32, tag="psne",
                                   name=f"psne{t}_{j}")
                    nc.tensor.transpose(ps, accS[:, j * 128:(j + 1) * 128],
                                        ident)
                    e_all = small.tile([128, 64], f32, tag="e_all")
                    nc.scalar.activation(e_all, ps[:, 0:64], func=act.Exp)
                    # expert half holds el/2, so tanh = 2*sigmoid(el)-1
                    th = small.tile([128, 64], f32, tag="th")
                    nc.scalar.activation(th, ps[:, 64:128], func=act.Tanh)
                    # top-8 selected in exp-space (monotone, so the same
                    # set), which needs no separate exp of the threshold and
                    # lets e_all start without waiting on a PSUM-bank handoff
                    tv = small.tile([128, 8], f32, tag="tv")
                    nc.vector.max(tv, e_all)              # top-8, descending
                    # g = exp(v) on the top-8 positions, exactly 0 elsewhere;
                    # zsum and the half-dot fall out of the fused accums
                    g = small.tile([128, 64], f32, tag="g")
                    nc.vector.scalar_tensor_tensor(
                        out=g, in0=e_all, scalar=tv[:, 7:8], in1=e_all,
                        op0=alu.is_ge, op1=alu.mult,
                        accum_out=zsum[:, j:j + 1])
                    scr = small.tile([128, 64], f32, tag="scr")
                    nc.vector.scalar_tensor_tensor(
                        out=scr, in0=g, scalar=0.5, in1=th,
                        op0=alu.mult, op1=alu.mult,
                        accum_out=s4h[:, j:j + 1])
                rz = small.tile([128, nj], f32, tag="rz")
                nc.vector.reciprocal(rz, zsum)
                # sigma = 0.5*tanh+0.5  =>  out = (0.5*s4)/zsum + 0.5
                if nj == 1:
                    nc.vector.tensor_scalar(
                        out=fin16[:, col0:col0 + 1], in0=s4h, scalar1=rz,
                        scalar2=0.5, op0=alu.mult, op1=alu.add)
                else:
                    fr = small.tile([128, nj], f32, tag="fr")
                    nc.vector.tensor_mul(fr, s4h, rz)
                    nc.vector.tensor_scalar(
                        out=fin16[:, col0:col0 + nj], in0=fr,
                        scalar1=0.5, scalar2=None, op0=alu.add)
                # Output DMAs: anything transferred before the x stream ends
                # delays the stream (one shared bandwidth device), so tiles
                # 0..NT-2 go out as ONE DMA whose read range spans all their
                # columns — it cannot fire until the second-to-last tile's
                # fin lands (~stream end), and the idle SP queue issues it
                # without touching the epilogue engines. The last tile's
                # single column rides Pool's SWDGE: cheapest fixed cost on
                # the final chain.
                if t == NT - 2:
                    nc.sync.dma_start(out=outp[:, 0:col0 + nj],
                                      in_=fin16[:, 0:col0 + nj])
                elif t == NT - 1:
                    nc.gpsimd.dma_start(out=outp[:, col0:col0 + nj],
                                        in_=fin16[:, col0:col0 + nj])

            pending = None
            for t in range(NT):
                bt = TILES[t]
                acc = psacc.tile([128, bt], f32, tag="acc", name=f"acc{t}")
                k = 0
                for c, ck in enumerate(CHUNKS[t]):
                    xk = xpool.tile([128, 16 * 512], f16, tag="xk")
                    xkv = xk.rearrange("p (a b) -> p a b", a=16 * 512 // bt,
                                       b=bt)
                    nc.sync.dma_start(out=xkv[:, 0:ck, :],
                                      in_=xts[t][:, k:k + ck, :])
                    for kc in range(ck):
                        nc.tensor.matmul(acc, lhsT=wt_sb[:, k + kc, :],
                                         rhs=xkv[:, kc, :],
                                         start=(k + kc == 0),
                                         stop=(k + kc == NK - 1))
                    k += ck
                    if c == 0 and pending is not None:
                        emit_epilogue(*pending)
                        pending = None
                pending = (t, acc)
            emit_epilogue(*pending)

    nc.compile()
    return nc


def get_program():
    if "prog" not in _cached:
        _cached["prog"] = _build_program()
    return _cached["prog"]


def make_in_maps(x, noise_w, noise_b, expert_w, expert_b):
    """Host-side sharding: per-core fp16 x slices + replicated fp16 weights.

    The expert weights/bias are halved so the on-chip tanh of the raw
    accumulator equals 2*sigmoid(expert_logit)-1.
    """
    w_comb = np.concatenate([noise_w, 0.5 * np.asarray(expert_w)],
                            axis=0).astype(np.float32)
    wt16 = w_comb.T.astype(np.float16)                       # [D, 128]
    wt = np.ascontiguousarray(
        wt16.reshape(NK, 128, 128).transpose(1, 0, 2))       # [128, NK, 128]
    bb = np.concatenate([noise_b, 0.5 * np.asarray(expert_b)]).astype(
        np.float32).reshape(128, 1)
    in_maps = []
    for c in range(NCORES):
        xs = x[c * BC:(c + 1) * BC, :].astype(np.float16)    # [BC, D]
        xsT = np.ascontiguousarray(xs.T)                     # [D, BC]
        im = {"wt": wt, "bb": bb}
        b0 = 0
        for t, bt in enumerate(TILES):
            # [p, nk, b]: contiguous per-partition blocks per tile
            im[f"xt{t}"] = np.ascontiguousarray(
                xsT[:, b0:b0 + bt].reshape(NK, 128, bt).transpose(1, 0, 2))
            b0 += bt
        in_maps.append(im)
    return in_maps


def kernel(x, noise, router_w, router_b, noise_w, noise_b, expert_w, expert_b,
           _trace=False):
    from concourse.bass_utils import run_bass_kernel_spmd

    x = np.asarray(x, dtype=np.float32)
    nc = get_program()
    in_maps = make_in_maps(x, np.asarray(noise_w), np.asarray(noise_b),
                           np.asarray(expert_w), np.asarray(expert_b))
    res = run_bass_kernel_spmd(nc, in_maps, core_ids=list(range(NCORES)),
                               trace=_trace)
    out = np.concatenate([r["out"] for r in res.results], axis=0)
    if _trace:
        kernel.last_results = res
    return out


# revision 27
# speedup vs baseline: 2.3658x; 1.0001x over previous
"""MoE logistic regression kernel for 8 Trainium2 NeuronCores.

Math (after dead-code elimination of the reference's unused router path):
    noise_logits = x @ noise_w.T + noise_b            # [B, E]
    top8 = top_k(noise_logits, 8)
    gates = softmax over the top-8 entries (others 0)
    expert = sigmoid(x @ expert_w.T + expert_b)       # [B, E]
    out[b] = sum_e gates[b,e] * expert[b,e]           # [B, 1]

Sharding: batch split 8 ways (2048 rows/core); weights replicated.

Implementation notes:
- Single-pass fp16 matmul (x and w rounded to fp16 on the host; fp32
  PSUM accumulate). Logit error ~4e-4 flips the top-8 set on only ~25
  of 16384 rows whose 8th/9th margin is that small; output l2 rel err
  ~1.2e-3, far under the 2e-2 gate. Halves DMA traffic and cuts PE
  work 3x vs an exact hi/lo split.
- noise_w/expert_w concatenated into one 128-wide stationary operand so
  x streams through the PE once for both matmuls.
- sigmoid(x) computed as 0.5*tanh(x/2)+0.5 with expert weights/bias
  pre-halved on the host: tanh and exp share one ACT function-table set
  so the kernel needs a single LoadActFuncSet, not 2x1283ns per tile.
- Batch tiles [512,512,512,256,128,128]: the taper keeps every
  epilogue except the last inside the DMA stream's shadow, and the
  final 128-row tile makes the last serial chain short. Tile t's
  epilogue instructions are emitted after tile t+1's first matmul
  chunk so the in-order PE queue never stalls on epilogue deps.
- Each transposed 128-row group gets its OWN PSUM bank: ScalarE and
  VectorE may only touch the same PSUM bank serially, so per-bank
  tiles let exp/tanh (ACT) run concurrently with Max8 (DVE) across
  groups. Top-8 selection runs entirely in exp-space: Max8 directly on
  exp(v) (monotone, so the same set, and self-consistent against
  activation-table rounding); gates and the gate*expert dot are two
  fused scalar_tensor_tensor ops whose accum_out gives zsum and 0.5*s4
  for free.
- Output needs no transpose: out rows c*128+p equal fin16[p, c], and a
  DRAM access pattern rearranged to [p][c] iterates in the same order
  as the SBUF source, so a strided DMA lands rows directly.
- x is staged host-side as [tile, partition, kchunk, col] fp16 so every
  DMA reads contiguous per-partition blocks (full 360GB/s); the last
  tile's trailing chunks shrink so the final matmuls start sooner.
"""

import sys

import numpy as np

if "/opt/trn_rl_repo" not in sys.path:
    sys.path.insert(0, "/opt/trn_rl_repo")

B, D, E, TOPK, NCORES = 16384, 4096, 64, 8, 8
BC = B // NCORES      # batch rows per core
NK = D // 128         # contraction chunks

TILES = [512, 512, 512, 256, 128, 128]   # batch rows per tile (sum = BC)
NT = len(TILES)
NOUT = BC // 128                         # output columns of fin16

# k-chunks per x DMA, per tile (sum = NK per tile)
CHUNKS = [
    [16, 16],
    [16, 16],
    [16, 16],
    [8, 8, 8, 8],
    [8, 8, 8, 8],
    [8, 8, 8, 4, 4],
]
W_CHUNKS = [8, 8, 8, 8]

_cached = {}


def _build_program():
    import concourse.bass as bass
    import concourse.tile as tile
    from concourse import bacc, mybir
    from concourse.masks import make_identity

    f32 = mybir.dt.float32
    f16 = mybir.dt.float16
    act = mybir.ActivationFunctionType
    alu = mybir.AluOpType

    nc = bacc.Bacc("TRN2", target_bir_lowering=False, debug=False)
    xts = [nc.dram_tensor(f"xt{t}", [128, NK, bt], f16,
                          kind="ExternalInput").ap()
           for t, bt in enumerate(TILES)]
    wt = nc.dram_tensor("wt", [128, NK, 128], f16, kind="ExternalInput").ap()
    bb = nc.dram_tensor("bb", [128, 1], f32, kind="ExternalInput").ap()
    out = nc.dram_tensor("out", [BC, 1], f32, kind="ExternalOutput").ap()

    with tile.TileContext(nc) as tc:
        with (
            tc.tile_pool(name="consts", bufs=1) as consts,
            tc.tile_pool(name="xpool", bufs=8) as xpool,
            tc.tile_pool(name="ep", bufs=2) as ep,
            tc.tile_pool(name="small", bufs=3) as small,
            tc.tile_pool(name="psacc", bufs=2, space=bass.MemorySpace.PSUM) as psacc,
            tc.tile_pool(name="pstr", bufs=5, space=bass.MemorySpace.PSUM) as pstr,
        ):
            # ---- constants ----
            bb_sb = consts.tile([128, 1], f32)
            nc.scalar.dma_start(out=bb_sb, in_=bb)
            wt_sb = consts.tile([128, NK, 128], f16)
            k0 = 0
            for wc in W_CHUNKS:
                nc.scalar.dma_start(out=wt_sb[:, k0:k0 + wc, :],
                                    in_=wt[:, k0:k0 + wc, :])
                k0 += wc
            ident = consts.tile([128, 128], f32)
            make_identity(nc, ident)
            # load the (single) ACT function set during the DMA phase; Tanh
            # and Exp both live in "exp_and_others"
            warm = consts.tile([1, 1], f32)
            nc.vector.memset(warm, 0.0)
            nc.scalar.add(warm, warm, bb_sb[0:1, :])
            nc.scalar.activation(warm, warm, func=act.Tanh)
            nc.scalar.activation(warm, warm, func=act.Exp)

            fin16 = consts.tile([128, NOUT], f32)
            # out rows c*128+p == fin16[p, c]: iterate DRAM as [p][c] and a
            # plain DMA from [128, c] SBUF lands rows with no transpose
            outp = out.rearrange("(c p) o -> p (c o)", p=128)   # [128, NOUT]

            def emit_epilogue(t, acc):
                bt = TILES[t]
                nj = bt // 128
                col0 = sum(TILES[:t]) // 128
                accS = ep.tile([128, bt], f32, tag="accS")
                if t == NT - 1:
                    # DVE add: off the ACT queue, which is still draining the
                    # previous tile's exps when the final accumulator lands
                    nc.vector.tensor_scalar_add(accS, acc, bb_sb)
                else:
                    nc.scalar.add(accS, acc, bb_sb)
                zsum = small.tile([128, nj], f32, tag="zsum")
                s4h = small.tile([128, nj], f32, tag="s4h")
                for j in range(nj):
                    # own PSUM bank per 128-row group: ACT and DVE readers
                    # of different groups may then run concurrently
                    ps = pstr.tile([128, 128], f32, tag="psne",
                                   name=f"psne{t}_{j}")
                    nc.tensor.transpose(ps, accS[:, j * 128:(j + 1) * 128],
                                        ident)
                    e_all = small.tile([128, 64], f32, tag="e_all")
                    nc.scalar.activation(e_all, ps[:, 0:64], func=act.Exp)
                    # expert half holds el/2, so tanh = 2*sigmoid(el)-1
                    th = small.tile([128, 64], f32, tag="th")
                    nc.scalar.activation(th, ps[:, 64:128], func=act.Tanh)
                    # top-8 selected in exp-space (monotone, so the same
                    # set), which needs no separate exp of the threshold and
                    # lets e_all start without waiting on a PSUM-bank handoff
                    tv = small.tile([128, 8], f32, tag="tv")
                    nc.vector.max(tv, e_all)              # top-8, descending
                    # g = exp(v) on the top-8 positions, exactly 0 elsewhere;
                    # zsum and the half-dot fall out of the fused accums
                    g = small.tile([128, 64], f32, tag="g")
                    nc.vector.scalar_tensor_tensor(
                        out=g, in0=e_all, scalar=tv[:, 7:8], in1=e_all,
                        op0=alu.is_ge, op1=alu.mult,
                        accum_out=zsum[:, j:j + 1])
                    scr = small.tile([128, 64], f32, tag="scr")
                    nc.vector.scalar_tensor_tensor(
                        out=scr, in0=g, scalar=0.5, in1=th,
                        op0=alu.mult, op1=alu.mult,
                        accum_out=s4h[:, j:j + 1])
                rz = small.tile([128, nj], f32, tag="rz")
                nc.vector.reciprocal(rz, zsum)
                # sigma = 0.5*tanh+0.5  =>  out = (0.5*s4)/zsum + 0.5
                if nj == 1:
                    nc.vector.tensor_scalar(
                        out=fin16[:, col0:col0 + 1], in0=s4h, scalar1=rz,
                        scalar2=0.5, op0=alu.mult, op1=alu.add)
                else:
                    fr = small.tile([128, nj], f32, tag="fr")
                    nc.vector.tensor_mul(fr, s4h, rz)
                    nc.vector.tensor_scalar(
                        out=fin16[:, col0:col0 + nj], in0=fr,
                        scalar1=0.5, scalar2=None, op0=alu.add)
                # Output DMAs: anything transferred before the x stream ends
                # delays the stream (one shared bandwidth device), so tiles
                # 0..NT-2 go out as ONE DMA whose read range spans all their
                # columns — it cannot fire until the second-to-last tile's
                # fin lands (~stream end), and the idle SP queue issues it
                # without touching the epilogue engines. The last tile's
                # single column rides Pool's SWDGE: cheapest fixed cost on
                # the final chain.
                if t == NT - 2:
                    nc.sync.dma_start(out=outp[:, 0:col0 + nj],
                                      in_=fin16[:, 0:col0 + nj])
                elif t == NT - 1:
                    nc.gpsimd.dma_start(out=outp[:, col0:col0 + nj],
                                        in_=fin16[:, col0:col0 + nj])

            pending = None
            for t in range(NT):
                bt = TILES[t]
                acc = psacc.tile([128, bt], f32, tag="acc", name=f"acc{t}")
                k = 0
                for c, ck in enumerate(CHUNKS[t]):
                    xk = xpool.tile([128, 16 * 512], f16, tag="xk")
                    xkv = xk.rearrange("p (a b) -> p a b", a=16 * 512 // bt,
                                       b=bt)
                    nc.sync.dma_start(out=xkv[:, 0:ck, :],
                                      in_=xts[t][:, k:k + ck, :])
                    for kc in range(ck):
                        nc.tensor.matmul(acc, lhsT=wt_sb[:, k + kc, :],
                                         rhs=xkv[:, kc, :],
                                         start=(k + kc == 0),
                                         stop=(k + kc == NK - 1))
                    k += ck
                    if c == 0 and pending is not None:
                        emit_epilogue(*pending)
                        pending = None
                pending = (t, acc)
            emit_epilogue(*pending)

    nc.compile()
    return nc


def get_program():
    if "prog" not in _cached:
        _cached["prog"] = _build_program()
    return _cached["prog"]


def make_in_maps(x, noise_w, noise_b, expert_w, expert_b):
    """Host-side sharding: per-core fp16 x slices + replicated fp16 weights.

    The expert weights/bias are halved so the on-chip tanh of the raw
    accumulator equals 2*sigmoid(expert_logit)-1.
    """
    w_comb = np.concatenate([noise_w, 0.5 * np.asarray(expert_w)],
                            axis=0).astype(np.float32)
    wt16 = w_comb.T.astype(np.float16)                       # [D, 128]
    wt = np.ascontiguousarray(
        wt16.reshape(NK, 128, 128).transpose(1, 0, 2))       # [128, NK, 128]
    bb = np.concatenate([noise_b, 0.5 * np.asarray(expert_b)]).astype(
        np.float32).reshape(128, 1)
    in_maps = []
    for c in range(NCORES):
        xs = x[c * BC:(c + 1) * BC, :].astype(np.float16)    # [BC, D]
        xsT = np.ascontiguousarray(xs.T)                     # [D, BC]
        im = {"wt": wt, "bb": bb}
        b0 = 0
        for t, bt in enumerate(TILES):
            # [p, nk, b]: contiguous per-partition blocks per tile
            im[f"xt{t}"] = np.ascontiguousarray(
                xsT[:, b0:b0 + bt].reshape(NK, 128, bt).transpose(1, 0, 2))
            b0 += bt
        in_maps.append(im)
    return in_maps


def kernel(x, noise, router_w, router_b, noise_w, noise_b, expert_w, expert_b,
           _trace=False):
    from concourse.bass_utils import run_bass_kernel_spmd

    x = np.asarray(x, dtype=np.float32)
    nc = get_program()
    in_maps = make_in_maps(x, np.asarray(noise_w), np.asarray(noise_b),
                           np.asarray(expert_w), np.asarray(expert_b))
    res = run_bass_kernel_spmd(nc, in_maps, core_ids=list(range(NCORES)),
                               trace=_trace)
    out = np.concatenate([r["out"] for r in res.results], axis=0)
    if _trace:
        kernel.last_results = res
    return out
